# revision 1
# baseline (speedup 1.0000x reference)
"""MoE feed-forward (top-1 routing, capacity 640, swiglu experts) on 8 trn2 cores.

Strategy (expert-parallel, per the sharding hint):
  * Host: router matmul/softmax/argmax + capacity-slot assignment (index
    plumbing, ~0.1% of FLOPs), gathers tokens per expert, pairs a heavy
    expert with a light one per core (greedy balance), 2 experts per core.
  * Device (Bass/Tile, per core): grouped GEMM  h = x @ W1  -> swiglu ->
    y = g @ W2, weighted by combine gates.  Matmuls in bf16 with fp32
    accumulate (bf16 keeps LDWEIGHTS off the critical path).  GEMM1
    computes hT [feat, tok] so GEMM2 needs no on-chip transpose.  Token
    tiles are sized to the actual routed token counts (rounded up to 128)
    instead of the worst-case capacity; the program is cached per
    (mt0, mt1) m-tile profile.
  * Host: scatter weighted expert outputs back to token order; dense
    fallback FFN applied only to dropped tokens (none at typical loads).
"""

import os
import sys

import numpy as np


def _ensure_concourse():
    try:
        import concourse.bass  # noqa: F401
    except Exception:
        for p in ("/opt/trn_rl_repo", "/root/.axon_site/_ro/trn_rl_repo"):
            if os.path.isdir(p) and p not in sys.path:
                sys.path.insert(0, p)
        import concourse.bass  # noqa: F401


# Problem constants (hardcoded per the task contract).
B, S, D, H, E = 4, 2048, 768, 3072, 16
N = B * S
C = 640  # capacity per expert (ceil(1.25 * N / E))
FALLBACK_W = 1.0
NCORES = 8
EL = E // NCORES  # experts per core = 2
KD = D // 128  # 6 k-tiles for GEMM1 contraction
FB = (2 * H) // 128  # 48 feature blocks of GEMM1 output
FP = FB // 2  # 24 swiglu pairs == k-tiles of GEMM2 contraction
KH = H // 128  # 24
MT = C // 128  # max token m-tiles per expert
DH = 384  # output d half-tile (2 x 384 = 768)

_NC_CACHE = {}  # (mt0, mt1) -> compiled Bass program
_WCACHE = {}  # weight reorder cache
LAST = None  # BassKernelResults of the most recent run (for profiling)


def _tok_tiles(pad):
    """Split a padded token count into moving-operand tiles (<=512)."""
    out, off = [], 0
    while pad - off > 512:
        out.append((off, 512))
        off += 512
    out.append((off, pad - off))
    return out


def _build_nc(mts):
    """Per-core Bass program: 2 expert slots with mts[s] token m-tiles each."""
    import concourse.bacc as bacc
    import concourse.mybir as mybir
    import concourse.tile as tile
    from contextlib import ExitStack

    f32 = mybir.dt.float32
    bf16 = mybir.dt.bfloat16
    AF = mybir.ActivationFunctionType
    ALU = mybir.AluOpType

    pads = [m * 128 for m in mts]
    tot = sum(pads)

    nc = bacc.Bacc("TRN2", target_bir_lowering=False)
    # Host-side layouts are pre-tiled so every DMA is 2D [128, contiguous].
    xt = nc.dram_tensor("xt", [128, KD * tot], bf16, kind="ExternalInput")
    w1r = nc.dram_tensor("w1r", [EL, FP, 128, 2 * KD * 128], bf16, kind="ExternalInput")
    w2t = nc.dram_tensor("w2t", [EL, 128, KH * D], bf16, kind="ExternalInput")
    b1t = nc.dram_tensor("b1t", [EL, 128, FB], f32, kind="ExternalInput")
    wce = nc.dram_tensor("wce", [EL, 128, MT], f32, kind="ExternalInput")
    y = nc.dram_tensor("y", [tot, D], f32, kind="ExternalOutput")

    with tile.TileContext(nc) as tc, ExitStack() as ctx:
        xp = ctx.enter_context(tc.tile_pool(name="xp", bufs=2))
        w2p = ctx.enter_context(tc.tile_pool(name="w2p", bufs=2))
        gp = ctx.enter_context(tc.tile_pool(name="gp", bufs=2))
        w1p = ctx.enter_context(tc.tile_pool(name="w1p", bufs=6))
        sap = ctx.enter_context(tc.tile_pool(name="sap", bufs=3))
        cst = ctx.enter_context(tc.tile_pool(name="cst", bufs=2))
        yp = ctx.enter_context(tc.tile_pool(name="yp", bufs=4))
        p1 = ctx.enter_context(tc.tile_pool(name="p1", bufs=3, space="PSUM"))
        p2 = ctx.enter_context(tc.tile_pool(name="p2", bufs=2, space="PSUM"))

        for e in range(EL):
            pad = pads[e]
            xoff = KD * pads[0] if e else 0
            tiles = _tok_tiles(pad)
            xsb = xp.tile([128, KD * pad], bf16, tag="x")
            # per-k chunks so the first matmul doesn't wait for the full load
            for k in range(KD):
                nc.gpsimd.dma_start(
                    xsb[:, k * pad : (k + 1) * pad],
                    xt[:, xoff + k * pad : xoff + (k + 1) * pad],
                )
            b1sb = cst.tile([128, FB], f32, tag="b1")
            nc.gpsimd.dma_start(b1sb[:], b1t[e, :, :])
            wcsb = cst.tile([128, MT], f32, tag="wc")
            nc.gpsimd.dma_start(wcsb[:], wce[e, :, :])

            gt = gp.tile([128, KH * pad], bf16, tag="g")

            # GEMM1 + swiglu: hT tiles [feat 128, tok <=512]
            for fp in range(FP):
                w1t = w1p.tile([128, 2 * KD * 128], bf16, tag="w1")
                nc.sync.dma_start(w1t[:], w1r[e, fp, :, :])
                w1a = w1t[:, : KD * 128]
                w1b = w1t[:, KD * 128 :]
                for toff, tn in tiles:
                    pa = p1.tile([128, tn], f32, tag="pa")
                    pb = p1.tile([128, tn], f32, tag="pb")
                    for k in range(KD):
                        nc.tensor.matmul(
                            pa[:],
                            lhsT=w1a[:, k * 128 : (k + 1) * 128],
                            rhs=xsb[:, k * pad + toff : k * pad + toff + tn],
                            start=(k == 0),
                            stop=(k == KD - 1),
                        )
                    for k in range(KD):
                        nc.tensor.matmul(
                            pb[:],
                            lhsT=w1b[:, k * 128 : (k + 1) * 128],
                            rhs=xsb[:, k * pad + toff : k * pad + toff + tn],
                            start=(k == 0),
                            stop=(k == KD - 1),
                        )
                    sa = sap.tile([128, tn], f32, tag="sa")
                    # silu(a + b1_a)
                    nc.scalar.activation(
                        sa[:], pa[:], AF.Silu, bias=b1sb[:, fp : fp + 1], scale=1.0
                    )
                    # g = (b + b1_b) * silu(...)
                    nc.vector.scalar_tensor_tensor(
                        out=gt[:, fp * pad + toff : fp * pad + toff + tn],
                        in0=pb[:],
                        scalar=b1sb[:, FP + fp : FP + fp + 1],
                        in1=sa[:],
                        op0=ALU.add,
                        op1=ALU.mult,
                    )

            # GEMM2: y[tok 128, d 384] = sum_k g[tok, h_k] @ W2[h_k, d]
            w2sb = w2p.tile([128, KH * D], bf16, tag="w2")
            nc.gpsimd.dma_start(w2sb[:], w2t[e, :, :])
            yoff = pads[0] if e else 0
            for m in range(mts[e]):
                for dh in range(2):
                    pt = p2.tile([128, DH], f32, tag="p2")
                    for k in range(KH):
                        nc.tensor.matmul(
                            pt[:],
                            lhsT=gt[:, k * pad + m * 128 : k * pad + m * 128 + 128],
                            rhs=w2sb[:, k * D + dh * DH : k * D + (dh + 1) * DH],
                            start=(k == 0),
                            stop=(k == KH - 1),
                        )
                    ysb = yp.tile([128, DH], f32, tag="y")
                    # weighted combine: y *= gate (per-token scalar); b2 is
                    # handled host-side (it is all zeros for this problem).
                    nc.scalar.activation(
                        ysb[:], pt[:], AF.Copy, bias=0.0, scale=wcsb[:, m : m + 1]
                    )
                    nc.gpsimd.dma_start(
                        y[
                            yoff + m * 128 : yoff + (m + 1) * 128,
                            dh * DH : (dh + 1) * DH,
                        ],
                        ysb[:],
                    )
    nc.compile()
    return nc


def _get_nc(mts):
    nc = _NC_CACHE.get(mts)
    if nc is None:
        nc = _NC_CACHE[mts] = _build_nc(mts)
    return nc


def _reorder_weights(W1, W2, b1):
    key = (W1.__array_interface__["data"][0], W2.__array_interface__["data"][0])
    hit = _WCACHE.get(key)
    if hit is not None:
        return hit
    import ml_dtypes

    W1 = np.ascontiguousarray(W1, dtype=np.float32)
    W2 = np.ascontiguousarray(W2, dtype=np.float32)
    b1 = np.ascontiguousarray(b1, dtype=np.float32)
    # W1 [E, D, 2H] -> [E, FB, 128p(d within k), KD*128(f)]
    w1f = (
        W1.reshape(E, KD, 128, FB, 128)
        .transpose(0, 3, 2, 1, 4)
        .reshape(E, FB, 128, KD * 128)
        .astype(ml_dtypes.bfloat16)
    )
    # combine swiglu pair (fp, fp+FP) into one contiguous block per DMA
    w1r = np.ascontiguousarray(np.concatenate([w1f[:, :FP], w1f[:, FP:]], axis=-1))
    # W2 [E, H, D] -> [E, 128p(h within k), KH*D]
    w2t = np.ascontiguousarray(
        W2.reshape(E, KH, 128, D)
        .transpose(0, 2, 1, 3)
        .reshape(E, 128, KH * D)
        .astype(ml_dtypes.bfloat16)
    )
    # b1 [E, 2H] -> [E, 128, FB]
    b1t = np.ascontiguousarray(b1.reshape(E, FB, 128).transpose(0, 2, 1))
    out = (w1r, w2t, b1t)
    _WCACHE.clear()
    _WCACHE[key] = out
    return out


def _route(x_flat, Wr):
    logits = x_flat @ np.ascontiguousarray(Wr, dtype=np.float32)  # [N, E]
    lmax = logits.max(axis=-1, keepdims=True)
    p = np.exp(logits - lmax)
    gates = p / p.sum(axis=-1, keepdims=True)
    expert = np.argmax(gates, axis=-1)
    # slot = occurrence index of each token within its expert's queue
    order = np.argsort(expert, kind="stable")
    sorted_e = expert[order]
    starts = np.searchsorted(sorted_e, np.arange(E))
    within = np.arange(N) - starts[sorted_e]
    slot = np.empty(N, np.int64)
    slot[order] = within
    kept = slot < C
    top_idx = np.zeros((C, E), np.int32)
    valid = np.zeros((C, E), np.float32)
    tok = np.arange(N, dtype=np.int32)
    top_idx[slot[kept], expert[kept]] = tok[kept]
    valid[slot[kept], expert[kept]] = 1.0
    w_ce = gates[top_idx, np.arange(E)[None, :]].astype(np.float32) * valid  # [C, E]
    n_kept = np.minimum(np.bincount(expert, minlength=E), C)  # [E]
    return gates, expert, kept, top_idx, valid, w_ce, n_kept


def kernel(x, Wr, W1, b1, W2, b2, W1f, b1f, W2f, b2f, _trace=False):
    global LAST
    _ensure_concourse()
    import ml_dtypes
    from concourse.bass_utils import run_bass_kernel_spmd

    x_flat = np.ascontiguousarray(np.asarray(x).reshape(N, D), dtype=np.float32)
    gates, expert, kept, top_idx, valid, w_ce, n_kept = _route(x_flat, np.asarray(Wr))
    w1r, w2t, b1t = _reorder_weights(np.asarray(W1), np.asarray(W2), np.asarray(b1))

    # Pair heavy experts with light ones (greedy balance); slot 0 = heavy.
    order = np.argsort(-n_kept, kind="stable")
    assign = [(int(order[i]), int(order[E - 1 - i])) for i in range(NCORES)]
    mt_of = [max(1, int(-(-n // 128))) for n in n_kept]
    mts = (
        max(mt_of[a] for a, _ in assign),
        max(mt_of[b] for _, b in assign),
    )
    pads = [m * 128 for m in mts]

    nc = _get_nc(mts)
    in_maps = []
    for c in range(NCORES):
        exps = assign[c]
        # gather + transpose tokens for each slot: [128, KD * pad]
        xparts = []
        for s, e in enumerate(exps):
            ids = top_idx[: n_kept[e], e]
            xg = np.zeros((pads[s], D), np.float32)
            xg[: len(ids)] = x_flat[ids]
            xparts.append(
                xg.reshape(pads[s], KD, 128)
                .transpose(2, 1, 0)
                .reshape(128, KD * pads[s])
            )
        xt_c = np.ascontiguousarray(
            np.concatenate(xparts, axis=1), dtype=ml_dtypes.bfloat16
        )
        wct = np.zeros((EL, 128, MT), np.float32)
        for s, e in enumerate(exps):
            w = np.zeros(pads[s], np.float32)
            w[: n_kept[e]] = w_ce[: n_kept[e], e]
            wct[s, :, : mts[s]] = w.reshape(mts[s], 128).T
        el = list(exps)
        in_maps.append(
            {
                "xt": xt_c,
                "w1r": np.ascontiguousarray(w1r[el]),
                "w2t": np.ascontiguousarray(w2t[el]),
                "b1t": np.ascontiguousarray(b1t[el]),
                "wce": wct,
            }
        )
    res = run_bass_kernel_spmd(nc, in_maps, list(range(NCORES)), trace=_trace)
    LAST = res

    # Combine: scatter weighted expert outputs back to token order.
    y_flat = np.zeros((N, D), np.float32)
    b2 = np.asarray(b2)
    add_b2 = bool(np.any(b2))
    for c in range(NCORES):
        yc = res.results[c]["y"]
        for s, e in enumerate(assign[c]):
            n = int(n_kept[e])
            ids = top_idx[:n, e]
            off = pads[0] if s else 0
            y_flat[ids] = yc[off : off + n]
            if add_b2:
                y_flat[ids] += w_ce[:n, e][:, None] * b2[e]

    # Dense fallback for fully-dropped tokens (rare; none at typical loads).
    dropped = ~kept
    if np.any(dropped):
        xd = x_flat[dropped]
        hf = xd @ np.asarray(W1f) + np.asarray(b1f)
        gf = (hf[:, :H] / (1.0 + np.exp(-hf[:, :H]))) * hf[:, H:]
        y_flat[dropped] += FALLBACK_W * (gf @ np.asarray(W2f) + np.asarray(b2f))

    return y_flat.reshape(B, S, D)



# revision 3
# speedup vs baseline: 1.0727x; 1.0727x over previous
"""MoE feed-forward (top-1 routing, capacity 640, swiglu experts) on 8 trn2 cores.

Strategy (expert-parallel, per the sharding hint):
  * Host: router matmul/softmax/argmax + capacity-slot assignment (index
    plumbing, ~0.1% of FLOPs), gathers tokens per expert, pairs a heavy
    expert with a light one per core (greedy balance), 2 experts per core.
  * Device (Bass/Tile, per core): grouped GEMM  h = x @ W1  -> swiglu ->
    yT = W2^T @ g, in bf16 with fp32 accumulate.  Both GEMMs keep tokens in
    the moving (free) dimension, so token counts are exact (rounded to 8)
    rather than padded to 128: GEMM1 produces hT [feat, tok], GEMM2
    produces yT [d, tok].  Combine-gate scaling and the scatter back to
    token order happen on the host, so no on-chip transpose is needed.
  * DMA pacing: W2 is streamed in 24 per-k chunks interleaved with the W1
    tile stream on the same (sync) queue so the bulk W2 load cannot starve
    the W1 tiles GEMM1 is consuming (this stall cost ~7us in the padded
    baseline).  First x chunk + first W1 tile are the first DMAs issued.
  * Host: scatter weighted expert outputs back to token order; dense
    fallback FFN applied only to dropped tokens (none at typical loads).
"""

import os
import sys

import numpy as np


def _ensure_concourse():
    try:
        import concourse.bass  # noqa: F401
    except Exception:
        for p in ("/opt/trn_rl_repo", "/root/.axon_site/_ro/trn_rl_repo"):
            if os.path.isdir(p) and p not in sys.path:
                sys.path.insert(0, p)
        import concourse.bass  # noqa: F401


# Problem constants (hardcoded per the task contract).
B, S, D, H, E = 4, 2048, 768, 3072, 16
N = B * S
C = 640  # capacity per expert (ceil(1.25 * N / E))
FALLBACK_W = 1.0
NCORES = 8
EL = E // NCORES  # experts per core = 2
KD = D // 128  # 6 k-tiles for GEMM1 contraction
FB = (2 * H) // 128  # 48 feature blocks of GEMM1 output
FP = FB // 2  # 24 swiglu pairs == k-tiles of GEMM2 contraction
KH = H // 128  # 24
DT = D // 128  # 6 output d-tiles of GEMM2

_NC_CACHE = {}  # (L0, L1) -> compiled Bass program
_WCACHE = {}  # weight reorder cache
LAST = None  # BassKernelResults of the most recent run (for profiling)


def _chunks(L):
    """Split token count L into near-equal moving chunks <= 512 (8-aligned)."""
    nch = -(-L // 512)
    base = -(-(-(-L // nch)) // 8) * 8
    out, off = [], 0
    for _ in range(nch - 1):
        out.append((off, base))
        off += base
    out.append((off, L - off))
    return out


def _build_nc(Ls):
    """Per-core Bass program: 2 expert slots with Ls[s] (8-aligned) tokens."""
    import concourse.bacc as bacc
    import concourse.mybir as mybir
    import concourse.tile as tile
    from contextlib import ExitStack

    f32 = mybir.dt.float32
    bf16 = mybir.dt.bfloat16
    AF = mybir.ActivationFunctionType
    ALU = mybir.AluOpType

    L0, L1 = Ls
    tot = L0 + L1

    nc = bacc.Bacc("TRN2", target_bir_lowering=False)
    # Host-side layouts are pre-tiled so every DMA is 2D [128, contiguous].
    xt = nc.dram_tensor("xt", [128, KD * tot], bf16, kind="ExternalInput")
    w1r = nc.dram_tensor("w1r", [EL, FP, 128, 2 * KD * 128], bf16, kind="ExternalInput")
    w2t = nc.dram_tensor("w2t", [EL, 128, KH * D], bf16, kind="ExternalInput")
    b1t = nc.dram_tensor("b1t", [EL, 128, FB], f32, kind="ExternalInput")
    y = nc.dram_tensor("y", [D, tot], bf16, kind="ExternalOutput")

    with tile.TileContext(nc) as tc, ExitStack() as ctx:
        xp = ctx.enter_context(tc.tile_pool(name="xp", bufs=2))
        w2p = ctx.enter_context(tc.tile_pool(name="w2p", bufs=2))
        gp = ctx.enter_context(tc.tile_pool(name="gp", bufs=2))
        w1p = ctx.enter_context(tc.tile_pool(name="w1p", bufs=6))
        sap = ctx.enter_context(tc.tile_pool(name="sap", bufs=3))
        cst = ctx.enter_context(tc.tile_pool(name="cst", bufs=2))
        yp = ctx.enter_context(tc.tile_pool(name="yp", bufs=4))
        p1 = ctx.enter_context(tc.tile_pool(name="p1", bufs=3, space="PSUM"))
        p2 = ctx.enter_context(tc.tile_pool(name="p2", bufs=2, space="PSUM"))

        for e in range(EL):
            L = Ls[e]
            xoff = KD * L0 if e else 0
            yoff = L0 if e else 0
            tiles = _chunks(L)
            xsb = xp.tile([128, KD * L], bf16, tag="x")
            # k=0 chunk rides the sync queue ahead of the first W1 tile so
            # the first matmul can start as early as possible.
            nc.sync.dma_start(xsb[:, :L], xt[:, xoff : xoff + L])
            for k in range(1, KD):
                nc.gpsimd.dma_start(
                    xsb[:, k * L : (k + 1) * L],
                    xt[:, xoff + k * L : xoff + (k + 1) * L],
                )
            b1sb = cst.tile([128, FB], f32, tag="b1")
            nc.gpsimd.dma_start(b1sb[:], b1t[e, :, :])

            gt = gp.tile([128, KH * L], bf16, tag="g")
            w2sb = w2p.tile([128, KH * D], bf16, tag="w2")

            # GEMM1 + swiglu: hT tiles [feat 128, tok chunk]
            for fp in range(FP):
                w1t = w1p.tile([128, 2 * KD * 128], bf16, tag="w1")
                nc.sync.dma_start(w1t[:], w1r[e, fp, :, :])
                # Trickle W2 through the same queue (2 chunks per fp from
                # fp=8) so it is resident before GEMM2 without ever
                # bursting ahead of the W1 stream.
                if 8 <= fp < 20:
                    j = 2 * (fp - 8)
                    nc.sync.dma_start(
                        w2sb[:, j * D : (j + 2) * D], w2t[e, :, j * D : (j + 2) * D]
                    )
                w1a = w1t[:, : KD * 128]
                w1b = w1t[:, KD * 128 :]
                for toff, tn in tiles:
                    pa = p1.tile([128, tn], f32, tag="pa")
                    pb = p1.tile([128, tn], f32, tag="pb")
                    for k in range(KD):
                        nc.tensor.matmul(
                            pa[:],
                            lhsT=w1a[:, k * 128 : (k + 1) * 128],
                            rhs=xsb[:, k * L + toff : k * L + toff + tn],
                            start=(k == 0),
                            stop=(k == KD - 1),
                        )
                    for k in range(KD):
                        nc.tensor.matmul(
                            pb[:],
                            lhsT=w1b[:, k * 128 : (k + 1) * 128],
                            rhs=xsb[:, k * L + toff : k * L + toff + tn],
                            start=(k == 0),
                            stop=(k == KD - 1),
                        )
                    sa = sap.tile([128, tn], f32, tag="sa")
                    # silu(a + b1_a)
                    nc.scalar.activation(
                        sa[:], pa[:], AF.Silu, bias=b1sb[:, fp : fp + 1], scale=1.0
                    )
                    # g = (b + b1_b) * silu(...)
                    nc.vector.scalar_tensor_tensor(
                        out=gt[:, fp * L + toff : fp * L + toff + tn],
                        in0=pb[:],
                        scalar=b1sb[:, FP + fp : FP + fp + 1],
                        in1=sa[:],
                        op0=ALU.add,
                        op1=ALU.mult,
                    )

            # GEMM2: yT[d 128, tok chunk] = sum_k W2[h_k, d]^T @ g[h_k, tok]
            # Tokens stay in the moving dim => no padding to 128 rows, and
            # the combine-gate scaling moves to the host scatter.
            for ci, (toff, tn) in enumerate(tiles):
                for dh in range(DT):
                    pt = p2.tile([128, tn], f32, tag="p2")
                    for k in range(KH):
                        nc.tensor.matmul(
                            pt[:],
                            lhsT=w2sb[:, k * D + dh * 128 : k * D + (dh + 1) * 128],
                            rhs=gt[:, k * L + toff : k * L + toff + tn],
                            start=(k == 0),
                            stop=(k == KH - 1),
                        )
                    ysb = yp.tile([128, tn], bf16, tag="y")
                    if dh % 2 == 0:
                        nc.scalar.copy(ysb[:], pt[:])
                    else:
                        nc.vector.tensor_scalar_mul(ysb[:], pt[:], 1.0)
                    nc.gpsimd.dma_start(
                        y[
                            dh * 128 : (dh + 1) * 128,
                            yoff + toff : yoff + toff + tn,
                        ],
                        ysb[:],
                    )
    nc.compile()
    return nc


def _get_nc(Ls):
    nc = _NC_CACHE.get(Ls)
    if nc is None:
        nc = _NC_CACHE[Ls] = _build_nc(Ls)
    return nc


def _reorder_weights(W1, W2, b1):
    key = (W1.__array_interface__["data"][0], W2.__array_interface__["data"][0])
    hit = _WCACHE.get(key)
    if hit is not None:
        return hit
    import ml_dtypes

    W1 = np.ascontiguousarray(W1, dtype=np.float32)
    W2 = np.ascontiguousarray(W2, dtype=np.float32)
    b1 = np.ascontiguousarray(b1, dtype=np.float32)
    # W1 [E, D, 2H] -> [E, FB, 128p(d within k), KD*128(f)]
    w1f = (
        W1.reshape(E, KD, 128, FB, 128)
        .transpose(0, 3, 2, 1, 4)
        .reshape(E, FB, 128, KD * 128)
        .astype(ml_dtypes.bfloat16)
    )
    # combine swiglu pair (fp, fp+FP) into one contiguous block per DMA
    w1r = np.ascontiguousarray(np.concatenate([w1f[:, :FP], w1f[:, FP:]], axis=-1))
    # W2 [E, H, D] -> [E, 128p(h within k), KH*D]
    w2t = np.ascontiguousarray(
        W2.reshape(E, KH, 128, D)
        .transpose(0, 2, 1, 3)
        .reshape(E, 128, KH * D)
        .astype(ml_dtypes.bfloat16)
    )
    # b1 [E, 2H] -> [E, 128, FB]
    b1t = np.ascontiguousarray(b1.reshape(E, FB, 128).transpose(0, 2, 1))
    out = (w1r, w2t, b1t)
    _WCACHE.clear()
    _WCACHE[key] = out
    return out


def _route(x_flat, Wr):
    logits = x_flat @ np.ascontiguousarray(Wr, dtype=np.float32)  # [N, E]
    lmax = logits.max(axis=-1, keepdims=True)
    p = np.exp(logits - lmax)
    gates = p / p.sum(axis=-1, keepdims=True)
    expert = np.argmax(gates, axis=-1)
    # slot = occurrence index of each token within its expert's queue
    order = np.argsort(expert, kind="stable")
    sorted_e = expert[order]
    starts = np.searchsorted(sorted_e, np.arange(E))
    within = np.arange(N) - starts[sorted_e]
    slot = np.empty(N, np.int64)
    slot[order] = within
    kept = slot < C
    top_idx = np.zeros((C, E), np.int32)
    valid = np.zeros((C, E), np.float32)
    tok = np.arange(N, dtype=np.int32)
    top_idx[slot[kept], expert[kept]] = tok[kept]
    valid[slot[kept], expert[kept]] = 1.0
    w_ce = gates[top_idx, np.arange(E)[None, :]].astype(np.float32) * valid  # [C, E]
    n_kept = np.minimum(np.bincount(expert, minlength=E), C)  # [E]
    return gates, expert, kept, top_idx, valid, w_ce, n_kept


def kernel(x, Wr, W1, b1, W2, b2, W1f, b1f, W2f, b2f, _trace=False):
    global LAST
    _ensure_concourse()
    import ml_dtypes
    from concourse.bass_utils import run_bass_kernel_spmd

    x_flat = np.ascontiguousarray(np.asarray(x).reshape(N, D), dtype=np.float32)
    gates, expert, kept, top_idx, valid, w_ce, n_kept = _route(x_flat, np.asarray(Wr))
    w1r, w2t, b1t = _reorder_weights(np.asarray(W1), np.asarray(W2), np.asarray(b1))

    # Pair heavy experts with light ones (greedy balance); slot 0 = heavy.
    order = np.argsort(-n_kept, kind="stable")
    assign = [(int(order[i]), int(order[E - 1 - i])) for i in range(NCORES)]
    # Slot shapes: exact max routed count per slot, rounded up to 8.
    Ls = tuple(
        max(64, -(-max(int(n_kept[a[s]]) for a in assign) // 8) * 8) for s in range(EL)
    )
    L0, L1 = Ls

    nc = _get_nc(Ls)
    in_maps = []
    for c in range(NCORES):
        exps = assign[c]
        # gather + transpose tokens for each slot: [128, KD * L]
        xparts = []
        for s, e in enumerate(exps):
            ids = top_idx[: n_kept[e], e]
            xg = np.zeros((Ls[s], D), np.float32)
            xg[: len(ids)] = x_flat[ids]
            xparts.append(
                xg.reshape(Ls[s], KD, 128).transpose(2, 1, 0).reshape(128, KD * Ls[s])
            )
        xt_c = np.ascontiguousarray(
            np.concatenate(xparts, axis=1), dtype=ml_dtypes.bfloat16
        )
        el = list(exps)
        in_maps.append(
            {
                "xt": xt_c,
                "w1r": np.ascontiguousarray(w1r[el]),
                "w2t": np.ascontiguousarray(w2t[el]),
                "b1t": np.ascontiguousarray(b1t[el]),
            }
        )
    res = run_bass_kernel_spmd(nc, in_maps, list(range(NCORES)), trace=_trace)
    LAST = res

    # Combine: gate-weight + scatter expert outputs back to token order.
    y_flat = np.zeros((N, D), np.float32)
    b2 = np.asarray(b2)
    add_b2 = bool(np.any(b2))
    for c in range(NCORES):
        yc = res.results[c]["y"]  # [D, L0+L1] bf16
        for s, e in enumerate(assign[c]):
            n = int(n_kept[e])
            ids = top_idx[:n, e]
            off = L0 if s else 0
            w = w_ce[:n, e][:, None]
            y_flat[ids] = w * yc[:, off : off + n].T.astype(np.float32)
            if add_b2:
                y_flat[ids] += w * b2[e]

    # Dense fallback for fully-dropped tokens (rare; none at typical loads).
    dropped = ~kept
    if np.any(dropped):
        xd = x_flat[dropped]
        hf = xd @ np.asarray(W1f) + np.asarray(b1f)
        gf = (hf[:, :H] / (1.0 + np.exp(-hf[:, :H]))) * hf[:, H:]
        y_flat[dropped] += FALLBACK_W * (gf @ np.asarray(W2f) + np.asarray(b2f))

    return y_flat.reshape(B, S, D)


# revision 4
# speedup vs baseline: 1.0921x; 1.0181x over previous
"""MoE feed-forward (top-1 routing, capacity 640, swiglu experts) on 8 trn2 cores.

Strategy (expert-parallel, per the sharding hint):
  * Host: router matmul/softmax/argmax + capacity-slot assignment (index
    plumbing, ~0.1% of FLOPs), gathers tokens per expert, pairs a heavy
    expert with a light one per core (greedy balance), 2 experts per core.
  * Device (Bass/Tile, per core): grouped GEMM  h = x @ W1  -> swiglu ->
    yT = W2^T @ g, in bf16 with fp32 accumulate.  Both GEMMs keep tokens in
    the moving (free) dimension, so token counts are exact (rounded to 8)
    rather than padded to 128: GEMM1 produces hT [feat, tok], GEMM2
    produces yT [d, tok].  Combine-gate scaling and the scatter back to
    token order happen on the host, so no on-chip transpose is needed.
  * Startup: x is laid out chunk-major so the first matmul needs only the
    first token-chunk of x plus half of the first W1 tile (~0.6 MB instead
    of 1.3 MB of HBM traffic); dummy matmuls on a zeroed tile warm the PE
    clock (HAM un-throttle) during the initial DMA wait.
  * DMA pacing: W2 is streamed in per-k chunks interleaved with the W1
    tile stream on the same (sync) queue so the bulk W2 load cannot starve
    the W1 tiles GEMM1 is consuming.
  * Host: scatter weighted expert outputs back to token order; dense
    fallback FFN applied only to dropped tokens (none at typical loads).
"""

import os
import sys

import numpy as np


def _ensure_concourse():
    try:
        import concourse.bass  # noqa: F401
    except Exception:
        for p in ("/opt/trn_rl_repo", "/root/.axon_site/_ro/trn_rl_repo"):
            if os.path.isdir(p) and p not in sys.path:
                sys.path.insert(0, p)
        import concourse.bass  # noqa: F401


# Problem constants (hardcoded per the task contract).
B, S, D, H, E = 4, 2048, 768, 3072, 16
N = B * S
C = 640  # capacity per expert (ceil(1.25 * N / E))
FALLBACK_W = 1.0
NCORES = 8
EL = E // NCORES  # experts per core = 2
KD = D // 128  # 6 k-tiles for GEMM1 contraction
FB = (2 * H) // 128  # 48 feature blocks of GEMM1 output
FP = FB // 2  # 24 swiglu pairs == k-tiles of GEMM2 contraction
KH = H // 128  # 24
DT = D // 128  # 6 output d-tiles of GEMM2
NWARM = 32  # dummy matmuls to pre-warm the PE clock gate

_NC_CACHE = {}  # (L0, L1) -> compiled Bass program
_WCACHE = {}  # weight reorder cache
LAST = None  # BassKernelResults of the most recent run (for profiling)


def _chunks(L):
    """Split token count L into near-equal moving chunks <= 512 (8-aligned)."""
    nch = -(-L // 512)
    base = -(-(-(-L // nch)) // 8) * 8
    out, off = [], 0
    for _ in range(nch - 1):
        out.append((off, base))
        off += base
    out.append((off, L - off))
    return out


def _build_nc(Ls):
    """Per-core Bass program: 2 expert slots with Ls[s] (8-aligned) tokens."""
    import concourse.bacc as bacc
    import concourse.mybir as mybir
    import concourse.tile as tile
    from contextlib import ExitStack

    f32 = mybir.dt.float32
    bf16 = mybir.dt.bfloat16
    AF = mybir.ActivationFunctionType
    ALU = mybir.AluOpType

    L0, L1 = Ls
    tot = L0 + L1

    nc = bacc.Bacc("TRN2", target_bir_lowering=False)
    # Host-side layouts are pre-tiled so every DMA is 2D [128, contiguous].
    # xt is chunk-major: for each expert slot, for each token chunk (c0, cn),
    # a [128, KD * cn] block (k-tiles within the chunk are contiguous).
    xt = nc.dram_tensor("xt", [128, KD * tot], bf16, kind="ExternalInput")
    w1r = nc.dram_tensor("w1r", [EL, FP, 128, 2 * KD * 128], bf16, kind="ExternalInput")
    w2t = nc.dram_tensor("w2t", [EL, 128, KH * D], bf16, kind="ExternalInput")
    b1t = nc.dram_tensor("b1t", [EL, 128, FB], f32, kind="ExternalInput")
    y = nc.dram_tensor("y", [D, tot], bf16, kind="ExternalOutput")

    with tile.TileContext(nc) as tc, ExitStack() as ctx:
        xp = ctx.enter_context(tc.tile_pool(name="xp", bufs=2))
        w2p = ctx.enter_context(tc.tile_pool(name="w2p", bufs=2))
        gp = ctx.enter_context(tc.tile_pool(name="gp", bufs=2))
        w1p = ctx.enter_context(tc.tile_pool(name="w1p", bufs=6))
        sap = ctx.enter_context(tc.tile_pool(name="sap", bufs=3))
        cst = ctx.enter_context(tc.tile_pool(name="cst", bufs=2))
        yp = ctx.enter_context(tc.tile_pool(name="yp", bufs=4))
        p1 = ctx.enter_context(tc.tile_pool(name="p1", bufs=3, space="PSUM"))
        p2 = ctx.enter_context(tc.tile_pool(name="p2", bufs=2, space="PSUM"))

        # Pre-warm the PE clock gate (HAM) with dummy matmuls on a zeroed
        # tile while the first x/W1 DMAs are in flight.  ~32 cold 128-col
        # matmuls = ~3.4us of PE activity, which un-throttles the clock to
        # 2.4 GHz right as the real matmul stream begins.
        zt = cst.tile([128, 128], bf16, tag="zt")
        nc.vector.memset(zt[:], 0.0)
        pz = p2.tile([128, 128], f32, tag="p2")
        for i in range(NWARM):
            nc.tensor.matmul(
                pz[:], lhsT=zt[:], rhs=zt[:], start=(i == 0), stop=(i == NWARM - 1)
            )

        for e in range(EL):
            L = Ls[e]
            xoff = KD * L0 if e else 0
            yoff = L0 if e else 0
            tiles = _chunks(L)
            xsb = xp.tile([128, KD * L], bf16, tag="x")
            # Chunk-major x blocks ride the sync queue, interleaved with the
            # W1 tile stream so the first matmuls are fed as early as
            # possible without starving the weight stream.
            nc.sync.dma_start(
                xsb[:, : KD * tiles[0][1]], xt[:, xoff : xoff + KD * tiles[0][1]]
            )
            b1sb = cst.tile([128, FB], f32, tag="b1")
            nc.gpsimd.dma_start(b1sb[:], b1t[e, :, :])

            gt = gp.tile([128, KH * L], bf16, tag="g")
            w2sb = w2p.tile([128, KH * D], bf16, tag="w2")

            # GEMM1 + swiglu: hT tiles [feat 128, tok chunk]
            for fp in range(FP):
                w1t = w1p.tile([128, 2 * KD * 128], bf16, tag="w1")
                if fp == 0:
                    # split the first tile so the a-half (needed by the very
                    # first accumulation group) lands sooner
                    nc.sync.dma_start(w1t[:, : KD * 128], w1r[e, 0, :, : KD * 128])
                    nc.sync.dma_start(w1t[:, KD * 128 :], w1r[e, 0, :, KD * 128 :])
                    for toff, tn in tiles[1:]:
                        nc.sync.dma_start(
                            xsb[:, KD * toff : KD * (toff + tn)],
                            xt[:, xoff + KD * toff : xoff + KD * (toff + tn)],
                        )
                else:
                    nc.sync.dma_start(w1t[:], w1r[e, fp, :, :])
                # Trickle W2 through the same queue (2 chunks per fp from
                # fp=8) so it is resident before GEMM2 without ever
                # bursting ahead of the W1 stream.
                if 8 <= fp < 20:
                    j = 2 * (fp - 8)
                    nc.sync.dma_start(
                        w2sb[:, j * D : (j + 2) * D], w2t[e, :, j * D : (j + 2) * D]
                    )
                w1a = w1t[:, : KD * 128]
                w1b = w1t[:, KD * 128 :]
                for toff, tn in tiles:
                    xb = xsb[:, KD * toff : KD * (toff + tn)]
                    pa = p1.tile([128, tn], f32, tag="pa")
                    pb = p1.tile([128, tn], f32, tag="pb")
                    for k in range(KD):
                        nc.tensor.matmul(
                            pa[:],
                            lhsT=w1a[:, k * 128 : (k + 1) * 128],
                            rhs=xb[:, k * tn : (k + 1) * tn],
                            start=(k == 0),
                            stop=(k == KD - 1),
                        )
                    for k in range(KD):
                        nc.tensor.matmul(
                            pb[:],
                            lhsT=w1b[:, k * 128 : (k + 1) * 128],
                            rhs=xb[:, k * tn : (k + 1) * tn],
                            start=(k == 0),
                            stop=(k == KD - 1),
                        )
                    sa = sap.tile([128, tn], f32, tag="sa")
                    # silu(a + b1_a)
                    nc.scalar.activation(
                        sa[:], pa[:], AF.Silu, bias=b1sb[:, fp : fp + 1], scale=1.0
                    )
                    # g = (b + b1_b) * silu(...)
                    nc.vector.scalar_tensor_tensor(
                        out=gt[:, fp * L + toff : fp * L + toff + tn],
                        in0=pb[:],
                        scalar=b1sb[:, FP + fp : FP + fp + 1],
                        in1=sa[:],
                        op0=ALU.add,
                        op1=ALU.mult,
                    )

            # GEMM2: yT[d 128, tok chunk] = sum_k W2[h_k, d]^T @ g[h_k, tok]
            # Tokens stay in the moving dim => no padding to 128 rows, and
            # the combine-gate scaling moves to the host scatter.
            last_e = e == EL - 1
            for ci, (toff, tn) in enumerate(tiles):
                last_c = last_e and ci == len(tiles) - 1
                for dh in range(DT):
                    pt = p2.tile([128, tn], f32, tag="p2")
                    for k in range(KH):
                        nc.tensor.matmul(
                            pt[:],
                            lhsT=w2sb[:, k * D + dh * 128 : k * D + (dh + 1) * 128],
                            rhs=gt[:, k * L + toff : k * L + toff + tn],
                            start=(k == 0),
                            stop=(k == KH - 1),
                        )
                    ysb = yp.tile([128, tn], bf16, tag="y")
                    ydst = y[dh * 128 : (dh + 1) * 128, yoff + toff : yoff + toff + tn]
                    if last_c and dh == DT - 1:
                        # Final drain: split across both compute engines and
                        # both DMA queues to halve the kernel tail.
                        h = (tn // 16) * 8
                        nc.vector.tensor_scalar_mul(ysb[:, :h], pt[:, :h], 1.0)
                        nc.scalar.copy(ysb[:, h:], pt[:, h:])
                        nc.gpsimd.dma_start(ydst[:, :h], ysb[:, :h])
                        nc.sync.dma_start(ydst[:, h:], ysb[:, h:])
                    else:
                        if dh % 2 == 0:
                            nc.scalar.copy(ysb[:], pt[:])
                        else:
                            nc.vector.tensor_scalar_mul(ysb[:], pt[:], 1.0)
                        nc.gpsimd.dma_start(ydst, ysb[:])
    nc.compile()
    return nc


def _get_nc(Ls):
    nc = _NC_CACHE.get(Ls)
    if nc is None:
        nc = _NC_CACHE[Ls] = _build_nc(Ls)
    return nc


def _reorder_weights(W1, W2, b1):
    key = (W1.__array_interface__["data"][0], W2.__array_interface__["data"][0])
    hit = _WCACHE.get(key)
    if hit is not None:
        return hit
    import ml_dtypes

    W1 = np.ascontiguousarray(W1, dtype=np.float32)
    W2 = np.ascontiguousarray(W2, dtype=np.float32)
    b1 = np.ascontiguousarray(b1, dtype=np.float32)
    # W1 [E, D, 2H] -> [E, FB, 128p(d within k), KD*128(f)]
    w1f = (
        W1.reshape(E, KD, 128, FB, 128)
        .transpose(0, 3, 2, 1, 4)
        .reshape(E, FB, 128, KD * 128)
        .astype(ml_dtypes.bfloat16)
    )
    # combine swiglu pair (fp, fp+FP) into one contiguous block per DMA
    w1r = np.ascontiguousarray(np.concatenate([w1f[:, :FP], w1f[:, FP:]], axis=-1))
    # W2 [E, H, D] -> [E, 128p(h within k), KH*D]
    w2t = np.ascontiguousarray(
        W2.reshape(E, KH, 128, D)
        .transpose(0, 2, 1, 3)
        .reshape(E, 128, KH * D)
        .astype(ml_dtypes.bfloat16)
    )
    # b1 [E, 2H] -> [E, 128, FB]
    b1t = np.ascontiguousarray(b1.reshape(E, FB, 128).transpose(0, 2, 1))
    out = (w1r, w2t, b1t)
    _WCACHE.clear()
    _WCACHE[key] = out
    return out


def _route(x_flat, Wr):
    logits = x_flat @ np.ascontiguousarray(Wr, dtype=np.float32)  # [N, E]
    lmax = logits.max(axis=-1, keepdims=True)
    p = np.exp(logits - lmax)
    gates = p / p.sum(axis=-1, keepdims=True)
    expert = np.argmax(gates, axis=-1)
    # slot = occurrence index of each token within its expert's queue
    order = np.argsort(expert, kind="stable")
    sorted_e = expert[order]
    starts = np.searchsorted(sorted_e, np.arange(E))
    within = np.arange(N) - starts[sorted_e]
    slot = np.empty(N, np.int64)
    slot[order] = within
    kept = slot < C
    top_idx = np.zeros((C, E), np.int32)
    valid = np.zeros((C, E), np.float32)
    tok = np.arange(N, dtype=np.int32)
    top_idx[slot[kept], expert[kept]] = tok[kept]
    valid[slot[kept], expert[kept]] = 1.0
    w_ce = gates[top_idx, np.arange(E)[None, :]].astype(np.float32) * valid  # [C, E]
    n_kept = np.minimum(np.bincount(expert, minlength=E), C)  # [E]
    return gates, expert, kept, top_idx, valid, w_ce, n_kept


def kernel(x, Wr, W1, b1, W2, b2, W1f, b1f, W2f, b2f, _trace=False):
    global LAST
    _ensure_concourse()
    import ml_dtypes
    from concourse.bass_utils import run_bass_kernel_spmd

    x_flat = np.ascontiguousarray(np.asarray(x).reshape(N, D), dtype=np.float32)
    gates, expert, kept, top_idx, valid, w_ce, n_kept = _route(x_flat, np.asarray(Wr))
    w1r, w2t, b1t = _reorder_weights(np.asarray(W1), np.asarray(W2), np.asarray(b1))

    # Pair heavy experts with light ones (greedy balance); slot 0 = heavy.
    order = np.argsort(-n_kept, kind="stable")
    assign = [(int(order[i]), int(order[E - 1 - i])) for i in range(NCORES)]
    # Slot shapes: exact max routed count per slot, rounded up to 8.
    Ls = tuple(
        max(64, -(-max(int(n_kept[a[s]]) for a in assign) // 8) * 8) for s in range(EL)
    )
    L0, L1 = Ls

    nc = _get_nc(Ls)
    in_maps = []
    for c in range(NCORES):
        exps = assign[c]
        # gather + transpose tokens for each slot, chunk-major: for each
        # token chunk (c0, cn) a [128, KD*cn] block
        xparts = []
        for s, e in enumerate(exps):
            ids = top_idx[: n_kept[e], e]
            xg = np.zeros((Ls[s], D), np.float32)
            xg[: len(ids)] = x_flat[ids]
            for c0, cn in _chunks(Ls[s]):
                xparts.append(
                    xg[c0 : c0 + cn]
                    .reshape(cn, KD, 128)
                    .transpose(2, 1, 0)
                    .reshape(128, KD * cn)
                )
        xt_c = np.ascontiguousarray(
            np.concatenate(xparts, axis=1), dtype=ml_dtypes.bfloat16
        )
        el = list(exps)
        in_maps.append(
            {
                "xt": xt_c,
                "w1r": np.ascontiguousarray(w1r[el]),
                "w2t": np.ascontiguousarray(w2t[el]),
                "b1t": np.ascontiguousarray(b1t[el]),
            }
        )
    res = run_bass_kernel_spmd(nc, in_maps, list(range(NCORES)), trace=_trace)
    LAST = res

    # Combine: gate-weight + scatter expert outputs back to token order.
    y_flat = np.zeros((N, D), np.float32)
    b2 = np.asarray(b2)
    add_b2 = bool(np.any(b2))
    for c in range(NCORES):
        yc = res.results[c]["y"]  # [D, L0+L1] bf16
        for s, e in enumerate(assign[c]):
            n = int(n_kept[e])
            ids = top_idx[:n, e]
            off = L0 if s else 0
            w = w_ce[:n, e][:, None]
            y_flat[ids] = w * yc[:, off : off + n].T.astype(np.float32)
            if add_b2:
                y_flat[ids] += w * b2[e]

    # Dense fallback for fully-dropped tokens (rare; none at typical loads).
    dropped = ~kept
    if np.any(dropped):
        xd = x_flat[dropped]
        hf = xd @ np.asarray(W1f) + np.asarray(b1f)
        gf = (hf[:, :H] / (1.0 + np.exp(-hf[:, :H]))) * hf[:, H:]
        y_flat[dropped] += FALLBACK_W * (gf @ np.asarray(W2f) + np.asarray(b2f))

    return y_flat.reshape(B, S, D)


# revision 10
# speedup vs baseline: 1.0964x; 1.0039x over previous
"""MoE feed-forward (top-1 routing, capacity 640, swiglu experts) on 8 trn2 cores.

Strategy (expert-parallel, per the sharding hint):
  * Host: router matmul/softmax/argmax + capacity-slot assignment (index
    plumbing, ~0.1% of FLOPs), gathers tokens per expert, pairs a heavy
    expert with a light one per core (greedy balance), 2 experts per core.
  * Device (Bass/Tile, per core): grouped GEMM  h = x @ W1  -> swiglu ->
    yT = W2^T @ g, in bf16 with fp32 accumulate.  Both GEMMs keep tokens in
    the moving (free) dimension, so token counts are exact (rounded to 8)
    rather than padded to 128: GEMM1 produces hT [feat, tok], GEMM2
    produces yT [d, tok].  Combine-gate scaling and the scatter back to
    token order happen on the host, so no on-chip transpose is needed.
  * Startup: x is laid out chunk-major so the first matmul needs only the
    first token-chunk of x plus half of the first W1 tile (~0.6 MB instead
    of 1.3 MB of HBM traffic); dummy matmuls on a zeroed tile warm the PE
    clock (HAM un-throttle) during the initial DMA wait.
  * DMA pacing: W2 is streamed in per-k chunks interleaved with the W1
    tile stream on the same (sync) queue so the bulk W2 load cannot starve
    the W1 tiles GEMM1 is consuming.
  * Host: scatter weighted expert outputs back to token order; dense
    fallback FFN applied only to dropped tokens (none at typical loads).
"""

import os
import sys

import numpy as np


def _ensure_concourse():
    try:
        import concourse.bass  # noqa: F401
    except Exception:
        for p in ("/opt/trn_rl_repo", "/root/.axon_site/_ro/trn_rl_repo"):
            if os.path.isdir(p) and p not in sys.path:
                sys.path.insert(0, p)
        import concourse.bass  # noqa: F401


# Problem constants (hardcoded per the task contract).
B, S, D, H, E = 4, 2048, 768, 3072, 16
N = B * S
C = 640  # capacity per expert (ceil(1.25 * N / E))
FALLBACK_W = 1.0
NCORES = 8
EL = E // NCORES  # experts per core = 2
KD = D // 128  # 6 k-tiles for GEMM1 contraction
FB = (2 * H) // 128  # 48 feature blocks of GEMM1 output
FP = FB // 2  # 24 swiglu pairs == k-tiles of GEMM2 contraction
KH = H // 128  # 24
DT = D // 128  # 6 output d-tiles of GEMM2
NWARM = 20  # dummy matmuls to pre-warm the PE clock gate

_NC_CACHE = {}  # (L0, L1) -> compiled Bass program
_WCACHE = {}  # weight reorder cache
LAST = None  # BassKernelResults of the most recent run (for profiling)


def _chunks(L):
    """Split token count L into near-equal moving chunks <= 512 (8-aligned)."""
    nch = -(-L // 512)
    base = -(-(-(-L // nch)) // 8) * 8
    out, off = [], 0
    for _ in range(nch - 1):
        out.append((off, base))
        off += base
    out.append((off, L - off))
    return out


def _g1_chunks(L):
    """GEMM1 chunking: a small 128-token head chunk first, so the very first
    accumulation groups need only ~0.4 MB of DMA before starting.  A 128-col
    matmul and a (L-128)-col matmul cost exactly what two L/2 matmuls do."""
    if L <= 512:
        return [(0, L)]
    return [(0, 128), (128, L - 128)]


def _build_nc(Ls):
    """Per-core Bass program: 2 expert slots with Ls[s] (8-aligned) tokens."""
    import concourse.bacc as bacc
    import concourse.mybir as mybir
    import concourse.tile as tile
    from contextlib import ExitStack

    f32 = mybir.dt.float32
    bf16 = mybir.dt.bfloat16
    AF = mybir.ActivationFunctionType
    ALU = mybir.AluOpType

    L0, L1 = Ls
    tot = L0 + L1

    nc = bacc.Bacc("TRN2", target_bir_lowering=False)
    # Host-side layouts are pre-tiled so every DMA is 2D [128, contiguous].
    # xt is chunk-major: for each expert slot, for each token chunk (c0, cn),
    # a [128, KD * cn] block (k-tiles within the chunk are contiguous).
    xt = nc.dram_tensor("xt", [128, KD * tot], bf16, kind="ExternalInput")
    w1r = nc.dram_tensor("w1r", [EL, FP, 128, 2 * KD * 128], bf16, kind="ExternalInput")
    w2t = nc.dram_tensor("w2t", [EL, 128, KH * D], bf16, kind="ExternalInput")
    b1t = nc.dram_tensor("b1t", [EL, 128, FB], f32, kind="ExternalInput")
    y = nc.dram_tensor("y", [D, tot], bf16, kind="ExternalOutput")

    with tile.TileContext(nc) as tc, ExitStack() as ctx:
        xp = ctx.enter_context(tc.tile_pool(name="xp", bufs=2))
        w2p = ctx.enter_context(tc.tile_pool(name="w2p", bufs=2))
        gp = ctx.enter_context(tc.tile_pool(name="gp", bufs=2))
        w1p = ctx.enter_context(tc.tile_pool(name="w1p", bufs=6))
        sap = ctx.enter_context(tc.tile_pool(name="sap", bufs=3))
        cst = ctx.enter_context(tc.tile_pool(name="cst", bufs=2))
        yp = ctx.enter_context(tc.tile_pool(name="yp", bufs=4))
        p1 = ctx.enter_context(tc.tile_pool(name="p1", bufs=3, space="PSUM"))
        p2 = ctx.enter_context(tc.tile_pool(name="p2", bufs=2, space="PSUM"))

        # Pre-warm the PE clock gate (HAM) with dummy matmuls on a zeroed
        # tile while the first x/W1 DMAs are in flight.  ~32 cold 128-col
        # matmuls = ~3.4us of PE activity, which un-throttles the clock to
        # 2.4 GHz right as the real matmul stream begins.
        zt = cst.tile([128, 128], bf16, tag="zt")
        nc.vector.memset(zt[:], 0.0)
        pz = p2.tile([128, 128], f32, tag="p2")
        for i in range(NWARM):
            nc.tensor.matmul(
                pz[:], lhsT=zt[:], rhs=zt[:], start=(i == 0), stop=(i == NWARM - 1)
            )

        for e in range(EL):
            L = Ls[e]
            xoff = KD * L0 if e else 0
            yoff = L0 if e else 0
            tiles = _g1_chunks(L)
            xsb = xp.tile([128, KD * L], bf16, tag="x")
            # Chunk-major x blocks ride the sync queue, interleaved with the
            # W1 tile stream so the first matmuls are fed as early as
            # possible without starving the weight stream.  Blocks > 256
            # tokens are split into two k-halves for finer pipelining.
            def _xload(toff, tn):
                lo = xoff + KD * toff
                if tn > 256:
                    half = (KD // 2) * tn
                    nc.sync.dma_start(
                        xsb[:, KD * toff : KD * toff + half], xt[:, lo : lo + half]
                    )
                    nc.sync.dma_start(
                        xsb[:, KD * toff + half : KD * (toff + tn)],
                        xt[:, lo + half : lo + KD * tn],
                    )
                else:
                    nc.sync.dma_start(
                        xsb[:, KD * toff : KD * (toff + tn)], xt[:, lo : lo + KD * tn]
                    )

            _xload(*tiles[0])
            b1sb = cst.tile([128, FB], f32, tag="b1")
            nc.gpsimd.dma_start(b1sb[:], b1t[e, :, :])

            gt = gp.tile([128, KH * L], bf16, tag="g")
            w2sb = w2p.tile([128, KH * D], bf16, tag="w2")

            # GEMM1 + swiglu: hT tiles [feat 128, tok chunk]
            for fp in range(FP):
                w1t = w1p.tile([128, 2 * KD * 128], bf16, tag="w1")
                if fp == 0:
                    # split the first tile so the a-half (needed by the very
                    # first accumulation group) lands sooner
                    nc.sync.dma_start(w1t[:, : KD * 128], w1r[e, 0, :, : KD * 128])
                    nc.sync.dma_start(w1t[:, KD * 128 :], w1r[e, 0, :, KD * 128 :])
                    for toff, tn in tiles[1:]:
                        _xload(toff, tn)
                else:
                    nc.sync.dma_start(w1t[:], w1r[e, fp, :, :])
                # Trickle W2 through the same queue (2 chunks per fp from
                # fp=8) so it is resident before GEMM2 without ever
                # bursting ahead of the W1 stream.
                if 8 <= fp < 20:
                    j = 2 * (fp - 8)
                    nc.sync.dma_start(
                        w2sb[:, j * D : (j + 2) * D], w2t[e, :, j * D : (j + 2) * D]
                    )
                w1a = w1t[:, : KD * 128]
                w1b = w1t[:, KD * 128 :]
                for toff, tn in tiles:
                    xb = xsb[:, KD * toff : KD * (toff + tn)]
                    pa = p1.tile([128, tn], f32, tag="pa")
                    pb = p1.tile([128, tn], f32, tag="pb")
                    for k in range(KD):
                        nc.tensor.matmul(
                            pa[:],
                            lhsT=w1a[:, k * 128 : (k + 1) * 128],
                            rhs=xb[:, k * tn : (k + 1) * tn],
                            start=(k == 0),
                            stop=(k == KD - 1),
                        )
                    for k in range(KD):
                        nc.tensor.matmul(
                            pb[:],
                            lhsT=w1b[:, k * 128 : (k + 1) * 128],
                            rhs=xb[:, k * tn : (k + 1) * tn],
                            start=(k == 0),
                            stop=(k == KD - 1),
                        )
                    sa = sap.tile([128, tn], f32, tag="sa")
                    # silu(a + b1_a)
                    nc.scalar.activation(
                        sa[:], pa[:], AF.Silu, bias=b1sb[:, fp : fp + 1], scale=1.0
                    )
                    # g = (b + b1_b) * silu(...)
                    nc.vector.scalar_tensor_tensor(
                        out=gt[:, fp * L + toff : fp * L + toff + tn],
                        in0=pb[:],
                        scalar=b1sb[:, FP + fp : FP + fp + 1],
                        in1=sa[:],
                        op0=ALU.add,
                        op1=ALU.mult,
                    )

            # GEMM2: yT[d 128, tok chunk] = sum_k W2[h_k, d]^T @ g[h_k, tok]
            # Tokens stay in the moving dim => no padding to 128 rows, and
            # the combine-gate scaling moves to the host scatter.
            last_e = e == EL - 1
            g2tiles = _chunks(L)
            for ci, (toff, tn) in enumerate(g2tiles):
                last_c = last_e and ci == len(g2tiles) - 1
                for dh in range(DT):
                    pt = p2.tile([128, tn], f32, tag="p2")
                    for k in range(KH):
                        nc.tensor.matmul(
                            pt[:],
                            lhsT=w2sb[:, k * D + dh * 128 : k * D + (dh + 1) * 128],
                            rhs=gt[:, k * L + toff : k * L + toff + tn],
                            start=(k == 0),
                            stop=(k == KH - 1),
                        )
                    ysb = yp.tile([128, tn], bf16, tag="y")
                    ydst = y[dh * 128 : (dh + 1) * 128, yoff + toff : yoff + toff + tn]
                    if last_c and dh == DT - 1:
                        # Final drain: split across both compute engines and
                        # both DMA queues to halve the kernel tail.
                        h = (tn // 16) * 8
                        nc.vector.tensor_scalar_mul(ysb[:, :h], pt[:, :h], 1.0)
                        nc.scalar.copy(ysb[:, h:], pt[:, h:])
                        nc.gpsimd.dma_start(ydst[:, :h], ysb[:, :h])
                        nc.sync.dma_start(ydst[:, h:], ysb[:, h:])
                    else:
                        if dh % 2 == 0:
                            nc.scalar.copy(ysb[:], pt[:])
                        else:
                            nc.vector.tensor_scalar_mul(ysb[:], pt[:], 1.0)
                        if last_c and dh == DT - 2:
                            # keep the gpsimd queue empty near the end so its
                            # final drain doesn't wait on a trailing DMA
                            nc.sync.dma_start(ydst, ysb[:])
                        else:
                            nc.gpsimd.dma_start(ydst, ysb[:])
    nc.compile()
    return nc


def _get_nc(Ls):
    nc = _NC_CACHE.get(Ls)
    if nc is None:
        nc = _NC_CACHE[Ls] = _build_nc(Ls)
    return nc


def _reorder_weights(W1, W2, b1):
    key = (W1.__array_interface__["data"][0], W2.__array_interface__["data"][0])
    hit = _WCACHE.get(key)
    if hit is not None:
        return hit
    import ml_dtypes

    W1 = np.ascontiguousarray(W1, dtype=np.float32)
    W2 = np.ascontiguousarray(W2, dtype=np.float32)
    b1 = np.ascontiguousarray(b1, dtype=np.float32)
    # W1 [E, D, 2H] -> [E, FB, 128p(d within k), KD*128(f)]
    w1f = (
        W1.reshape(E, KD, 128, FB, 128)
        .transpose(0, 3, 2, 1, 4)
        .reshape(E, FB, 128, KD * 128)
        .astype(ml_dtypes.bfloat16)
    )
    # combine swiglu pair (fp, fp+FP) into one contiguous block per DMA
    w1r = np.ascontiguousarray(np.concatenate([w1f[:, :FP], w1f[:, FP:]], axis=-1))
    # W2 [E, H, D] -> [E, 128p(h within k), KH*D]
    w2t = np.ascontiguousarray(
        W2.reshape(E, KH, 128, D)
        .transpose(0, 2, 1, 3)
        .reshape(E, 128, KH * D)
        .astype(ml_dtypes.bfloat16)
    )
    # b1 [E, 2H] -> [E, 128, FB]
    b1t = np.ascontiguousarray(b1.reshape(E, FB, 128).transpose(0, 2, 1))
    out = (w1r, w2t, b1t)
    _WCACHE.clear()
    _WCACHE[key] = out
    return out


def _route(x_flat, Wr):
    logits = x_flat @ np.ascontiguousarray(Wr, dtype=np.float32)  # [N, E]
    lmax = logits.max(axis=-1, keepdims=True)
    p = np.exp(logits - lmax)
    gates = p / p.sum(axis=-1, keepdims=True)
    expert = np.argmax(gates, axis=-1)
    # slot = occurrence index of each token within its expert's queue
    order = np.argsort(expert, kind="stable")
    sorted_e = expert[order]
    starts = np.searchsorted(sorted_e, np.arange(E))
    within = np.arange(N) - starts[sorted_e]
    slot = np.empty(N, np.int64)
    slot[order] = within
    kept = slot < C
    top_idx = np.zeros((C, E), np.int32)
    valid = np.zeros((C, E), np.float32)
    tok = np.arange(N, dtype=np.int32)
    top_idx[slot[kept], expert[kept]] = tok[kept]
    valid[slot[kept], expert[kept]] = 1.0
    w_ce = gates[top_idx, np.arange(E)[None, :]].astype(np.float32) * valid  # [C, E]
    n_kept = np.minimum(np.bincount(expert, minlength=E), C)  # [E]
    return gates, expert, kept, top_idx, valid, w_ce, n_kept


def kernel(x, Wr, W1, b1, W2, b2, W1f, b1f, W2f, b2f, _trace=False):
    global LAST
    _ensure_concourse()
    import ml_dtypes
    from concourse.bass_utils import run_bass_kernel_spmd

    x_flat = np.ascontiguousarray(np.asarray(x).reshape(N, D), dtype=np.float32)
    gates, expert, kept, top_idx, valid, w_ce, n_kept = _route(x_flat, np.asarray(Wr))
    w1r, w2t, b1t = _reorder_weights(np.asarray(W1), np.asarray(W2), np.asarray(b1))

    # Pair heavy experts with light ones (greedy balance); slot 0 = heavy.
    order = np.argsort(-n_kept, kind="stable")
    assign = [(int(order[i]), int(order[E - 1 - i])) for i in range(NCORES)]
    # Slot shapes: exact max routed count per slot, rounded up to 8.
    Ls = tuple(
        max(64, -(-max(int(n_kept[a[s]]) for a in assign) // 8) * 8) for s in range(EL)
    )
    L0, L1 = Ls

    nc = _get_nc(Ls)
    in_maps = []
    for c in range(NCORES):
        exps = assign[c]
        # gather + transpose tokens for each slot, chunk-major: for each
        # token chunk (c0, cn) a [128, KD*cn] block
        xparts = []
        for s, e in enumerate(exps):
            ids = top_idx[: n_kept[e], e]
            xg = np.zeros((Ls[s], D), np.float32)
            xg[: len(ids)] = x_flat[ids]
            for c0, cn in _g1_chunks(Ls[s]):
                xparts.append(
                    xg[c0 : c0 + cn]
                    .reshape(cn, KD, 128)
                    .transpose(2, 1, 0)
                    .reshape(128, KD * cn)
                )
        xt_c = np.ascontiguousarray(
            np.concatenate(xparts, axis=1), dtype=ml_dtypes.bfloat16
        )
        el = list(exps)
        in_maps.append(
            {
                "xt": xt_c,
                "w1r": np.ascontiguousarray(w1r[el]),
                "w2t": np.ascontiguousarray(w2t[el]),
                "b1t": np.ascontiguousarray(b1t[el]),
            }
        )
    res = run_bass_kernel_spmd(nc, in_maps, list(range(NCORES)), trace=_trace)
    LAST = res

    # Combine: gate-weight + scatter expert outputs back to token order.
    y_flat = np.zeros((N, D), np.float32)
    b2 = np.asarray(b2)
    add_b2 = bool(np.any(b2))
    for c in range(NCORES):
        yc = res.results[c]["y"]  # [D, L0+L1] bf16
        for s, e in enumerate(assign[c]):
            n = int(n_kept[e])
            ids = top_idx[:n, e]
            off = L0 if s else 0
            w = w_ce[:n, e][:, None]
            y_flat[ids] = w * yc[:, off : off + n].T.astype(np.float32)
            if add_b2:
                y_flat[ids] += w * b2[e]

    # Dense fallback for fully-dropped tokens (rare; none at typical loads).
    dropped = ~kept
    if np.any(dropped):
        xd = x_flat[dropped]
        hf = xd @ np.asarray(W1f) + np.asarray(b1f)
        gf = (hf[:, :H] / (1.0 + np.exp(-hf[:, :H]))) * hf[:, H:]
        y_flat[dropped] += FALLBACK_W * (gf @ np.asarray(W2f) + np.asarray(b2f))

    return y_flat.reshape(B, S, D)


# revision 11
# speedup vs baseline: 1.1014x; 1.0046x over previous
"""MoE feed-forward (top-1 routing, capacity 640, swiglu experts) on 8 trn2 cores.

Strategy (expert-parallel, per the sharding hint):
  * Host: router matmul/softmax/argmax + capacity-slot assignment (index
    plumbing, ~0.1% of FLOPs), gathers tokens per expert, pairs a heavy
    expert with a light one per core (greedy balance), 2 experts per core.
  * Device (Bass/Tile, per core): grouped GEMM  h = x @ W1  -> swiglu ->
    yT = W2^T @ g, in bf16 with fp32 accumulate.  Both GEMMs keep tokens in
    the moving (free) dimension, so token counts are exact (rounded to 8)
    rather than padded to 128: GEMM1 produces hT [feat, tok], GEMM2
    produces yT [d, tok].  Combine-gate scaling and the scatter back to
    token order happen on the host, so no on-chip transpose is needed.
  * Startup: each dma_start costs ~1us serialized (descriptor-gen +
    doorbell), so the critical first data (x token-chunk 0 + W1 tile 0 +
    its biases) is packed into ONE prologue DMA; b1 biases ride inside
    every W1 tile (bitcast bf16 pairs) so no tiny-descriptor bias DMA jams
    the queues; dummy matmuls on a zeroed tile warm the PE clock gate
    (HAM un-throttle) during the initial DMA wait.
  * DMA pacing: W2 is streamed in per-k chunks interleaved with the W1
    tile stream on the same (sync) queue so the bulk W2 load cannot starve
    the W1 tiles GEMM1 is consuming.
  * Tail: the very last GEMM2 accumulation is split into two half-width
    PSUM groups drained on different engines/queues to shorten the
    end-of-kernel chain.
  * Host: scatter weighted expert outputs back to token order; dense
    fallback FFN applied only to dropped tokens (none at typical loads).
"""

import os
import sys

import numpy as np


def _ensure_concourse():
    try:
        import concourse.bass  # noqa: F401
    except Exception:
        for p in ("/opt/trn_rl_repo", "/root/.axon_site/_ro/trn_rl_repo"):
            if os.path.isdir(p) and p not in sys.path:
                sys.path.insert(0, p)
        import concourse.bass  # noqa: F401


# Problem constants (hardcoded per the task contract).
B, S, D, H, E = 4, 2048, 768, 3072, 16
N = B * S
C = 640  # capacity per expert (ceil(1.25 * N / E))
FALLBACK_W = 1.0
NCORES = 8
EL = E // NCORES  # experts per core = 2
KD = D // 128  # 6 k-tiles for GEMM1 contraction
FB = (2 * H) // 128  # 48 feature blocks of GEMM1 output
FP = FB // 2  # 24 swiglu pairs == k-tiles of GEMM2 contraction
KH = H // 128  # 24
DT = D // 128  # 6 output d-tiles of GEMM2
W1W = 2 * KD * 128  # 1536 weight columns of one W1 tile
W1T = W1W + 4  # + 2 fp32 bias columns packed as 4 bf16
NWARM = 26  # dummy matmuls to pre-warm the PE clock gate

_NC_CACHE = {}  # (L0, L1) -> compiled Bass program
_WCACHE = {}  # weight reorder cache
LAST = None  # BassKernelResults of the most recent run (for profiling)


def _chunks(L):
    """Split token count L into near-equal moving chunks <= 512 (8-aligned)."""
    nch = -(-L // 512)
    base = -(-(-(-L // nch)) // 8) * 8
    out, off = [], 0
    for _ in range(nch - 1):
        out.append((off, base))
        off += base
    out.append((off, L - off))
    return out


def _g1_chunks(L, first):
    """GEMM1 chunking.  The first expert gets a 256-token head chunk that
    rides the prologue DMA; an n-col + (L-n)-col matmul pair costs exactly
    what two L/2-col matmuls do, so this is free."""
    if first:
        if L <= 256:
            return [(0, L)]
        return [(0, 256), (256, L - 256)]
    if L <= 512:
        return [(0, L)]
    return [(0, 128), (128, L - 128)]


def _build_nc(Ls):
    """Per-core Bass program: 2 expert slots with Ls[s] (8-aligned) tokens."""
    import concourse.bacc as bacc
    import concourse.mybir as mybir
    import concourse.tile as tile
    from contextlib import ExitStack

    f32 = mybir.dt.float32
    bf16 = mybir.dt.bfloat16
    AF = mybir.ActivationFunctionType
    ALU = mybir.AluOpType

    L0, L1 = Ls
    tot = L0 + L1
    g1t0 = _g1_chunks(L0, True)
    pro_x = KD * g1t0[0][1]  # x columns in the prologue pack

    nc = bacc.Bacc("TRN2", target_bir_lowering=False)
    # Host-side layouts are pre-tiled so every DMA is 2D [128, contiguous].
    # pro packs expert-0's first x token-chunk + W1 tile 0 (incl. biases).
    # xt is chunk-major: for each expert slot, for each remaining token
    # chunk (c0, cn), a [128, KD * cn] block.
    pro = nc.dram_tensor("pro", [128, pro_x + W1T], bf16, kind="ExternalInput")
    xt = nc.dram_tensor("xt", [128, KD * tot - pro_x], bf16, kind="ExternalInput")
    w1r = nc.dram_tensor("w1r", [EL, FP, 128, W1T], bf16, kind="ExternalInput")
    w2t = nc.dram_tensor("w2t", [EL, 128, KH * D], bf16, kind="ExternalInput")
    y = nc.dram_tensor("y", [D, tot], bf16, kind="ExternalOutput")

    with tile.TileContext(nc) as tc, ExitStack() as ctx:
        prop = ctx.enter_context(tc.tile_pool(name="prop", bufs=1))
        xp = ctx.enter_context(tc.tile_pool(name="xp", bufs=2))
        w2p = ctx.enter_context(tc.tile_pool(name="w2p", bufs=2))
        gp = ctx.enter_context(tc.tile_pool(name="gp", bufs=2))
        w1p = ctx.enter_context(tc.tile_pool(name="w1p", bufs=6))
        sap = ctx.enter_context(tc.tile_pool(name="sap", bufs=3))
        cst = ctx.enter_context(tc.tile_pool(name="cst", bufs=1))
        yp = ctx.enter_context(tc.tile_pool(name="yp", bufs=4))
        p1 = ctx.enter_context(tc.tile_pool(name="p1", bufs=3, space="PSUM"))
        p2 = ctx.enter_context(tc.tile_pool(name="p2", bufs=2, space="PSUM"))

        # Prologue pack: first on the sync queue.
        pro_sb = prop.tile([128, pro_x + W1T], bf16, tag="pro")
        nc.sync.dma_start(pro_sb[:], pro[:])

        # Pre-warm the PE clock gate (HAM) with dummy matmuls on a zeroed
        # tile while the first DMAs are in flight: ~2.8us of PE activity
        # un-throttles the clock to 2.4 GHz as the real stream begins.
        zt = cst.tile([128, 128], bf16, tag="zt")
        nc.vector.memset(zt[:], 0.0)
        pz = p2.tile([128, 128], f32, tag="p2")
        for i in range(NWARM):
            nc.tensor.matmul(
                pz[:], lhsT=zt[:], rhs=zt[:], start=(i == 0), stop=(i == NWARM - 1)
            )

        xoff = 0  # running column offset into xt
        for e in range(EL):
            L = Ls[e]
            yoff = L0 if e else 0
            tiles = _g1_chunks(L, e == 0)
            # x blocks: expert 0's first chunk lives in the prologue pack;
            # everything else is a [128, KD*cn] chunk-major block in xt.
            xaps = []
            rest = [t for t in tiles]
            if e == 0:
                xaps.append(pro_sb[:, :pro_x])
                rest = tiles[1:]
            if rest:
                xw = sum(KD * cn for _, cn in rest)
                xsb = xp.tile([128, xw], bf16, tag="x")
                boff = 0
                for _, cn in rest:
                    xaps.append(xsb[:, boff : boff + KD * cn])
                    nc.sync.dma_start(
                        xsb[:, boff : boff + KD * cn],
                        xt[:, xoff : xoff + KD * cn],
                    )
                    boff += KD * cn
                    xoff += KD * cn

            gt = gp.tile([128, KH * L], bf16, tag="g")
            w2sb = w2p.tile([128, KH * D], bf16, tag="w2")

            # GEMM1 + swiglu: hT tiles [feat 128, tok chunk]
            for fp in range(FP):
                if e == 0 and fp == 0:
                    w1t = pro_sb[:, pro_x:]
                else:
                    w1t = w1p.tile([128, W1T], bf16, tag="w1")
                    nc.sync.dma_start(w1t[:], w1r[e, fp, :, :])
                    w1t = w1t[:]
                # Trickle W2 through the same queue (2 chunks per fp from
                # fp=8) so it is resident before GEMM2 without ever
                # bursting ahead of the W1 stream.
                if 8 <= fp < 20:
                    j = 2 * (fp - 8)
                    nc.sync.dma_start(
                        w2sb[:, j * D : (j + 2) * D], w2t[e, :, j * D : (j + 2) * D]
                    )
                w1a = w1t[:, : KD * 128]
                w1b = w1t[:, KD * 128 : W1W]
                bia = w1t[:, W1W : W1W + 2].bitcast(f32)
                bib = w1t[:, W1W + 2 : W1W + 4].bitcast(f32)
                for ci, (toff, tn) in enumerate(tiles):
                    xb = xaps[ci]
                    pa = p1.tile([128, tn], f32, tag="pa")
                    pb = p1.tile([128, tn], f32, tag="pb")
                    for k in range(KD):
                        nc.tensor.matmul(
                            pa[:],
                            lhsT=w1a[:, k * 128 : (k + 1) * 128],
                            rhs=xb[:, k * tn : (k + 1) * tn],
                            start=(k == 0),
                            stop=(k == KD - 1),
                        )
                    for k in range(KD):
                        nc.tensor.matmul(
                            pb[:],
                            lhsT=w1b[:, k * 128 : (k + 1) * 128],
                            rhs=xb[:, k * tn : (k + 1) * tn],
                            start=(k == 0),
                            stop=(k == KD - 1),
                        )
                    sa = sap.tile([128, tn], f32, tag="sa")
                    # silu(a + b1_a)
                    nc.scalar.activation(sa[:], pa[:], AF.Silu, bias=bia, scale=1.0)
                    # g = (b + b1_b) * silu(...)
                    nc.vector.scalar_tensor_tensor(
                        out=gt[:, fp * L + toff : fp * L + toff + tn],
                        in0=pb[:],
                        scalar=bib,
                        in1=sa[:],
                        op0=ALU.add,
                        op1=ALU.mult,
                    )

            # GEMM2: yT[d 128, tok chunk] = sum_k W2[h_k, d]^T @ g[h_k, tok]
            # Tokens stay in the moving dim => no padding to 128 rows, and
            # the combine-gate scaling moves to the host scatter.
            last_e = e == EL - 1
            g2tiles = _chunks(L)

            def _g2(pt, toff, tn, dh):
                for k in range(KH):
                    nc.tensor.matmul(
                        pt[:],
                        lhsT=w2sb[:, k * D + dh * 128 : k * D + (dh + 1) * 128],
                        rhs=gt[:, k * L + toff : k * L + toff + tn],
                        start=(k == 0),
                        stop=(k == KH - 1),
                    )

            for ci, (toff, tn) in enumerate(g2tiles):
                last_c = last_e and ci == len(g2tiles) - 1
                for dh in range(DT):
                    ydst = y[dh * 128 : (dh + 1) * 128, yoff + toff : yoff + toff + tn]
                    if last_c and dh == DT - 1:
                        # Final tile: two half-width accumulation groups so
                        # the drain of the first half overlaps the matmuls
                        # of the second, and only a half-width copy + DMA
                        # remain after the very last matmul.
                        h = (tn // 16) * 8
                        ptA = p2.tile([128, h], f32, tag="p2")
                        _g2(ptA, toff, h, dh)
                        ysbA = yp.tile([128, h], bf16, tag="y")
                        nc.vector.tensor_scalar_mul(ysbA[:], ptA[:], 1.0)
                        nc.gpsimd.dma_start(ydst[:, :h], ysbA[:])
                        ptB = p2.tile([128, tn - h], f32, tag="p2")
                        _g2(ptB, toff + h, tn - h, dh)
                        ysbB = yp.tile([128, tn - h], bf16, tag="y")
                        nc.scalar.copy(ysbB[:], ptB[:])
                        nc.sync.dma_start(ydst[:, h:], ysbB[:])
                    else:
                        pt = p2.tile([128, tn], f32, tag="p2")
                        _g2(pt, toff, tn, dh)
                        ysb = yp.tile([128, tn], bf16, tag="y")
                        if dh % 2 == 0:
                            nc.scalar.copy(ysb[:], pt[:])
                        else:
                            nc.vector.tensor_scalar_mul(ysb[:], pt[:], 1.0)
                        if last_c and dh == DT - 2:
                            # keep the gpsimd queue empty near the end
                            nc.sync.dma_start(ydst, ysb[:])
                        else:
                            nc.gpsimd.dma_start(ydst, ysb[:])
    nc.compile()
    return nc


def _get_nc(Ls):
    nc = _NC_CACHE.get(Ls)
    if nc is None:
        nc = _NC_CACHE[Ls] = _build_nc(Ls)
    return nc


def _reorder_weights(W1, W2, b1):
    key = (W1.__array_interface__["data"][0], W2.__array_interface__["data"][0])
    hit = _WCACHE.get(key)
    if hit is not None:
        return hit
    import ml_dtypes

    W1 = np.ascontiguousarray(W1, dtype=np.float32)
    W2 = np.ascontiguousarray(W2, dtype=np.float32)
    b1 = np.ascontiguousarray(b1, dtype=np.float32)
    # W1 [E, D, 2H] -> [E, FB, 128p(d within k), KD*128(f)]
    w1f = (
        W1.reshape(E, KD, 128, FB, 128)
        .transpose(0, 3, 2, 1, 4)
        .reshape(E, FB, 128, KD * 128)
        .astype(ml_dtypes.bfloat16)
    )
    # swiglu pair (fp, fp+FP) in one block per DMA + b1 pair packed as
    # 2 fp32 (= 4 bf16) trailing columns
    b1a = b1[:, :H].reshape(E, FP, 128, 1)
    b1b = b1[:, H:].reshape(E, FP, 128, 1)
    baug = np.ascontiguousarray(np.concatenate([b1a, b1b], axis=-1)).view(
        ml_dtypes.bfloat16
    )  # [E, FP, 128, 4]
    w1r = np.ascontiguousarray(
        np.concatenate([w1f[:, :FP], w1f[:, FP:], baug], axis=-1)
    )  # [E, FP, 128, W1T]
    # W2 [E, H, D] -> [E, 128p(h within k), KH*D]
    w2t = np.ascontiguousarray(
        W2.reshape(E, KH, 128, D)
        .transpose(0, 2, 1, 3)
        .reshape(E, 128, KH * D)
        .astype(ml_dtypes.bfloat16)
    )
    out = (w1r, w2t)
    _WCACHE.clear()
    _WCACHE[key] = out
    return out


def _route(x_flat, Wr):
    logits = x_flat @ np.ascontiguousarray(Wr, dtype=np.float32)  # [N, E]
    lmax = logits.max(axis=-1, keepdims=True)
    p = np.exp(logits - lmax)
    gates = p / p.sum(axis=-1, keepdims=True)
    expert = np.argmax(gates, axis=-1)
    # slot = occurrence index of each token within its expert's queue
    order = np.argsort(expert, kind="stable")
    sorted_e = expert[order]
    starts = np.searchsorted(sorted_e, np.arange(E))
    within = np.arange(N) - starts[sorted_e]
    slot = np.empty(N, np.int64)
    slot[order] = within
    kept = slot < C
    top_idx = np.zeros((C, E), np.int32)
    valid = np.zeros((C, E), np.float32)
    tok = np.arange(N, dtype=np.int32)
    top_idx[slot[kept], expert[kept]] = tok[kept]
    valid[slot[kept], expert[kept]] = 1.0
    w_ce = gates[top_idx, np.arange(E)[None, :]].astype(np.float32) * valid  # [C, E]
    n_kept = np.minimum(np.bincount(expert, minlength=E), C)  # [E]
    return gates, expert, kept, top_idx, valid, w_ce, n_kept


def kernel(x, Wr, W1, b1, W2, b2, W1f, b1f, W2f, b2f, _trace=False):
    global LAST
    _ensure_concourse()
    import ml_dtypes
    from concourse.bass_utils import run_bass_kernel_spmd

    x_flat = np.ascontiguousarray(np.asarray(x).reshape(N, D), dtype=np.float32)
    gates, expert, kept, top_idx, valid, w_ce, n_kept = _route(x_flat, np.asarray(Wr))
    w1r, w2t = _reorder_weights(np.asarray(W1), np.asarray(W2), np.asarray(b1))

    # Pair heavy experts with light ones (greedy balance); slot 0 = heavy.
    order = np.argsort(-n_kept, kind="stable")
    assign = [(int(order[i]), int(order[E - 1 - i])) for i in range(NCORES)]
    # Slot shapes: exact max routed count per slot, rounded up to 8.
    Ls = tuple(
        max(64, -(-max(int(n_kept[a[s]]) for a in assign) // 8) * 8) for s in range(EL)
    )
    L0, L1 = Ls

    nc = _get_nc(Ls)
    in_maps = []
    for c in range(NCORES):
        exps = assign[c]
        # gather + transpose tokens for each slot, chunk-major: for each
        # token chunk (c0, cn) a [128, KD*cn] block
        xparts = []
        for s, e in enumerate(exps):
            ids = top_idx[: n_kept[e], e]
            xg = np.zeros((Ls[s], D), np.float32)
            xg[: len(ids)] = x_flat[ids]
            for c0, cn in _g1_chunks(Ls[s], s == 0):
                xparts.append(
                    xg[c0 : c0 + cn]
                    .reshape(cn, KD, 128)
                    .transpose(2, 1, 0)
                    .reshape(128, KD * cn)
                    .astype(ml_dtypes.bfloat16)
                )
        pro_c = np.ascontiguousarray(
            np.concatenate([xparts[0], w1r[exps[0], 0]], axis=1)
        )
        xt_c = np.ascontiguousarray(np.concatenate(xparts[1:], axis=1))
        el = list(exps)
        in_maps.append(
            {
                "pro": pro_c,
                "xt": xt_c,
                "w1r": np.ascontiguousarray(w1r[el]),
                "w2t": np.ascontiguousarray(w2t[el]),
            }
        )
    res = run_bass_kernel_spmd(nc, in_maps, list(range(NCORES)), trace=_trace)
    LAST = res

    # Combine: gate-weight + scatter expert outputs back to token order.
    y_flat = np.zeros((N, D), np.float32)
    b2 = np.asarray(b2)
    add_b2 = bool(np.any(b2))
    for c in range(NCORES):
        yc = res.results[c]["y"]  # [D, L0+L1] bf16
        for s, e in enumerate(assign[c]):
            n = int(n_kept[e])
            ids = top_idx[:n, e]
            off = L0 if s else 0
            w = w_ce[:n, e][:, None]
            y_flat[ids] = w * yc[:, off : off + n].T.astype(np.float32)
            if add_b2:
                y_flat[ids] += w * b2[e]

    # Dense fallback for fully-dropped tokens (rare; none at typical loads).
    dropped = ~kept
    if np.any(dropped):
        xd = x_flat[dropped]
        hf = xd @ np.asarray(W1f) + np.asarray(b1f)
        gf = (hf[:, :H] / (1.0 + np.exp(-hf[:, :H]))) * hf[:, H:]
        y_flat[dropped] += FALLBACK_W * (gf @ np.asarray(W2f) + np.asarray(b2f))

    return y_flat.reshape(B, S, D)


# revision 12
# speedup vs baseline: 1.1051x; 1.0034x over previous
"""MoE feed-forward (top-1 routing, capacity 640, swiglu experts) on 8 trn2 cores.

Strategy (expert-parallel, per the sharding hint):
  * Host: router matmul/softmax/argmax + capacity-slot assignment (index
    plumbing, ~0.1% of FLOPs), gathers tokens per expert, pairs a heavy
    expert with a light one per core (greedy balance), 2 experts per core.
  * Device (Bass/Tile, per core): grouped GEMM  h = x @ W1  -> swiglu ->
    yT = W2^T @ g, in bf16 with fp32 accumulate.  Both GEMMs keep tokens in
    the moving (free) dimension, so token counts are exact (rounded to 8)
    rather than padded to 128: GEMM1 produces hT [feat, tok], GEMM2
    produces yT [d, tok].  Combine-gate scaling and the scatter back to
    token order happen on the host, so no on-chip transpose is needed.
  * Startup: each dma_start costs ~1us serialized (descriptor-gen +
    doorbell), so the critical first data (x token-chunk 0 + W1 tile 0 +
    its biases) is packed into ONE prologue DMA; b1 biases ride inside
    every W1 tile (bitcast bf16 pairs) so no tiny-descriptor bias DMA jams
    the queues; dummy matmuls on a zeroed tile warm the PE clock gate
    (HAM un-throttle) during the initial DMA wait.
  * DMA pacing: W2 is streamed in per-k chunks interleaved with the W1
    tile stream on the same (sync) queue so the bulk W2 load cannot starve
    the W1 tiles GEMM1 is consuming.
  * Tail: the very last GEMM2 accumulation is split into two half-width
    PSUM groups drained on different engines/queues to shorten the
    end-of-kernel chain.
  * Host: scatter weighted expert outputs back to token order; dense
    fallback FFN applied only to dropped tokens (none at typical loads).
"""

import os
import sys

import numpy as np


def _ensure_concourse():
    try:
        import concourse.bass  # noqa: F401
    except Exception:
        for p in ("/opt/trn_rl_repo", "/root/.axon_site/_ro/trn_rl_repo"):
            if os.path.isdir(p) and p not in sys.path:
                sys.path.insert(0, p)
        import concourse.bass  # noqa: F401


# Problem constants (hardcoded per the task contract).
B, S, D, H, E = 4, 2048, 768, 3072, 16
N = B * S
C = 640  # capacity per expert (ceil(1.25 * N / E))
FALLBACK_W = 1.0
NCORES = 8
EL = E // NCORES  # experts per core = 2
KD = D // 128  # 6 k-tiles for GEMM1 contraction
FB = (2 * H) // 128  # 48 feature blocks of GEMM1 output
FP = FB // 2  # 24 swiglu pairs == k-tiles of GEMM2 contraction
KH = H // 128  # 24
DT = D // 128  # 6 output d-tiles of GEMM2
W1W = 2 * KD * 128  # 1536 weight columns of one W1 tile
W1T = W1W + 4  # + 2 fp32 bias columns packed as 4 bf16
NWARM = 44  # dummy matmuls to pre-warm the PE clock gate

_NC_CACHE = {}  # (L0, L1) -> compiled Bass program
_WCACHE = {}  # weight reorder cache
LAST = None  # BassKernelResults of the most recent run (for profiling)


def _chunks(L):
    """Split token count L into near-equal moving chunks <= 512 (8-aligned)."""
    nch = -(-L // 512)
    base = -(-(-(-L // nch)) // 8) * 8
    out, off = [], 0
    for _ in range(nch - 1):
        out.append((off, base))
        off += base
    out.append((off, L - off))
    return out


def _g1_chunks(L, first):
    """GEMM1 chunking.  The first expert gets a 256-token head chunk that
    rides the prologue DMA; an n-col + (L-n)-col matmul pair costs exactly
    what two L/2-col matmuls do, so this is free."""
    if first:
        if L <= 256:
            return [(0, L)]
        return [(0, 256), (256, L - 256)]
    if L <= 512:
        return [(0, L)]
    return [(0, 128), (128, L - 128)]


def _build_nc(Ls):
    """Per-core Bass program: 2 expert slots with Ls[s] (8-aligned) tokens."""
    import concourse.bacc as bacc
    import concourse.mybir as mybir
    import concourse.tile as tile
    from contextlib import ExitStack

    f32 = mybir.dt.float32
    bf16 = mybir.dt.bfloat16
    AF = mybir.ActivationFunctionType
    ALU = mybir.AluOpType

    L0, L1 = Ls
    tot = L0 + L1
    g1t0 = _g1_chunks(L0, True)
    pro_x = KD * g1t0[0][1]  # x columns in the prologue pack

    nc = bacc.Bacc("TRN2", target_bir_lowering=False)
    # Host-side layouts are pre-tiled so every DMA is 2D [128, contiguous].
    # pro packs expert-0's first x token-chunk + W1 tile 0 (incl. biases).
    # xt is chunk-major: for each expert slot, for each remaining token
    # chunk (c0, cn), a [128, KD * cn] block.
    pro = nc.dram_tensor("pro", [128, pro_x + W1T], bf16, kind="ExternalInput")
    xt = nc.dram_tensor("xt", [128, KD * tot - pro_x], bf16, kind="ExternalInput")
    w1r = nc.dram_tensor("w1r", [EL, FP, 128, W1T], bf16, kind="ExternalInput")
    w2t = nc.dram_tensor("w2t", [EL, 128, KH * D], bf16, kind="ExternalInput")
    y = nc.dram_tensor("y", [D, tot], bf16, kind="ExternalOutput")

    with tile.TileContext(nc) as tc, ExitStack() as ctx:
        prop = ctx.enter_context(tc.tile_pool(name="prop", bufs=1))
        xp = ctx.enter_context(tc.tile_pool(name="xp", bufs=2))
        w2p = ctx.enter_context(tc.tile_pool(name="w2p", bufs=2))
        gp = ctx.enter_context(tc.tile_pool(name="gp", bufs=2))
        w1p = ctx.enter_context(tc.tile_pool(name="w1p", bufs=6))
        sap = ctx.enter_context(tc.tile_pool(name="sap", bufs=3))
        cst = ctx.enter_context(tc.tile_pool(name="cst", bufs=1))
        yp = ctx.enter_context(tc.tile_pool(name="yp", bufs=4))
        p1 = ctx.enter_context(tc.tile_pool(name="p1", bufs=3, space="PSUM"))
        p2 = ctx.enter_context(tc.tile_pool(name="p2", bufs=2, space="PSUM"))

        # Prologue pack: first on the sync queue.
        pro_sb = prop.tile([128, pro_x + W1T], bf16, tag="pro")
        nc.sync.dma_start(pro_sb[:], pro[:])

        # Pre-warm the PE clock gate (HAM) with dummy matmuls on a zeroed
        # tile while the first DMAs are in flight: ~2.8us of PE activity
        # un-throttles the clock to 2.4 GHz as the real stream begins.
        zt = cst.tile([128, 128], bf16, tag="zt")
        nc.vector.memset(zt[:], 0.0)
        pz = p2.tile([128, 128], f32, tag="p2")
        for i in range(NWARM):
            nc.tensor.matmul(
                pz[:], lhsT=zt[:], rhs=zt[:], start=(i == 0), stop=(i == NWARM - 1)
            )

        xoff = 0  # running column offset into xt
        for e in range(EL):
            L = Ls[e]
            yoff = L0 if e else 0
            tiles = _g1_chunks(L, e == 0)
            # x blocks: expert 0's first chunk lives in the prologue pack;
            # everything else is a [128, KD*cn] chunk-major block in xt.
            xaps = []
            rest = [t for t in tiles]
            if e == 0:
                xaps.append(pro_sb[:, :pro_x])
                rest = tiles[1:]
            if rest:
                xw = sum(KD * cn for _, cn in rest)
                xsb = xp.tile([128, xw], bf16, tag="x")
                boff = 0
                for _, cn in rest:
                    xaps.append(xsb[:, boff : boff + KD * cn])
                    nc.sync.dma_start(
                        xsb[:, boff : boff + KD * cn],
                        xt[:, xoff : xoff + KD * cn],
                    )
                    boff += KD * cn
                    xoff += KD * cn

            gt = gp.tile([128, KH * L], bf16, tag="g")
            w2sb = w2p.tile([128, KH * D], bf16, tag="w2")

            # GEMM1 + swiglu: hT tiles [feat 128, tok chunk]
            for fp in range(FP):
                if e == 0 and fp == 0:
                    w1t = pro_sb[:, pro_x:]
                else:
                    w1t = w1p.tile([128, W1T], bf16, tag="w1")
                    nc.sync.dma_start(w1t[:], w1r[e, fp, :, :])
                    w1t = w1t[:]
                # Trickle W2 through the same queue (2 chunks per fp from
                # fp=8) so it is resident before GEMM2 without ever
                # bursting ahead of the W1 stream.
                if 8 <= fp < 20:
                    j = 2 * (fp - 8)
                    nc.sync.dma_start(
                        w2sb[:, j * D : (j + 2) * D], w2t[e, :, j * D : (j + 2) * D]
                    )
                w1a = w1t[:, : KD * 128]
                w1b = w1t[:, KD * 128 : W1W]
                bia = w1t[:, W1W : W1W + 2].bitcast(f32)
                bib = w1t[:, W1W + 2 : W1W + 4].bitcast(f32)
                for ci, (toff, tn) in enumerate(tiles):
                    xb = xaps[ci]
                    pa = p1.tile([128, tn], f32, tag="pa")
                    pb = p1.tile([128, tn], f32, tag="pb")
                    for k in range(KD):
                        nc.tensor.matmul(
                            pa[:],
                            lhsT=w1a[:, k * 128 : (k + 1) * 128],
                            rhs=xb[:, k * tn : (k + 1) * tn],
                            start=(k == 0),
                            stop=(k == KD - 1),
                        )
                    for k in range(KD):
                        nc.tensor.matmul(
                            pb[:],
                            lhsT=w1b[:, k * 128 : (k + 1) * 128],
                            rhs=xb[:, k * tn : (k + 1) * tn],
                            start=(k == 0),
                            stop=(k == KD - 1),
                        )
                    sa = sap.tile([128, tn], f32, tag="sa")
                    # silu(a + b1_a)
                    nc.scalar.activation(sa[:], pa[:], AF.Silu, bias=bia, scale=1.0)
                    # g = (b + b1_b) * silu(...)
                    nc.vector.scalar_tensor_tensor(
                        out=gt[:, fp * L + toff : fp * L + toff + tn],
                        in0=pb[:],
                        scalar=bib,
                        in1=sa[:],
                        op0=ALU.add,
                        op1=ALU.mult,
                    )

            # GEMM2: yT[d 128, tok chunk] = sum_k W2[h_k, d]^T @ g[h_k, tok]
            # Tokens stay in the moving dim => no padding to 128 rows, and
            # the combine-gate scaling moves to the host scatter.
            last_e = e == EL - 1
            g2tiles = _chunks(L)

            def _g2(pt, toff, tn, dh):
                for k in range(KH):
                    nc.tensor.matmul(
                        pt[:],
                        lhsT=w2sb[:, k * D + dh * 128 : k * D + (dh + 1) * 128],
                        rhs=gt[:, k * L + toff : k * L + toff + tn],
                        start=(k == 0),
                        stop=(k == KH - 1),
                    )

            for ci, (toff, tn) in enumerate(g2tiles):
                last_c = last_e and ci == len(g2tiles) - 1
                for dh in range(DT):
                    ydst = y[dh * 128 : (dh + 1) * 128, yoff + toff : yoff + toff + tn]
                    if last_c and dh == DT - 1:
                        # Final tile: two half-width accumulation groups so
                        # the drain of the first half overlaps the matmuls
                        # of the second, and only a half-width copy + DMA
                        # remain after the very last matmul.
                        h = (tn // 16) * 8
                        ptA = p2.tile([128, h], f32, tag="p2")
                        _g2(ptA, toff, h, dh)
                        ysbA = yp.tile([128, h], bf16, tag="y")
                        nc.vector.tensor_scalar_mul(ysbA[:], ptA[:], 1.0)
                        nc.gpsimd.dma_start(ydst[:, :h], ysbA[:])
                        ptB = p2.tile([128, tn - h], f32, tag="p2")
                        _g2(ptB, toff + h, tn - h, dh)
                        ysbB = yp.tile([128, tn - h], bf16, tag="y")
                        nc.scalar.copy(ysbB[:], ptB[:])
                        nc.sync.dma_start(ydst[:, h:], ysbB[:])
                    else:
                        pt = p2.tile([128, tn], f32, tag="p2")
                        _g2(pt, toff, tn, dh)
                        ysb = yp.tile([128, tn], bf16, tag="y")
                        if dh % 2 == 0:
                            nc.scalar.copy(ysb[:], pt[:])
                        else:
                            nc.vector.tensor_scalar_mul(ysb[:], pt[:], 1.0)
                        if last_c and dh == DT - 2:
                            # keep the gpsimd queue empty near the end
                            nc.sync.dma_start(ydst, ysb[:])
                        else:
                            nc.gpsimd.dma_start(ydst, ysb[:])
    nc.compile()
    return nc


def _get_nc(Ls):
    nc = _NC_CACHE.get(Ls)
    if nc is None:
        nc = _NC_CACHE[Ls] = _build_nc(Ls)
    return nc


def _reorder_weights(W1, W2, b1):
    key = (W1.__array_interface__["data"][0], W2.__array_interface__["data"][0])
    hit = _WCACHE.get(key)
    if hit is not None:
        return hit
    import ml_dtypes

    W1 = np.ascontiguousarray(W1, dtype=np.float32)
    W2 = np.ascontiguousarray(W2, dtype=np.float32)
    b1 = np.ascontiguousarray(b1, dtype=np.float32)
    # W1 [E, D, 2H] -> [E, FB, 128p(d within k), KD*128(f)]
    w1f = (
        W1.reshape(E, KD, 128, FB, 128)
        .transpose(0, 3, 2, 1, 4)
        .reshape(E, FB, 128, KD * 128)
        .astype(ml_dtypes.bfloat16)
    )
    # swiglu pair (fp, fp+FP) in one block per DMA + b1 pair packed as
    # 2 fp32 (= 4 bf16) trailing columns
    b1a = b1[:, :H].reshape(E, FP, 128, 1)
    b1b = b1[:, H:].reshape(E, FP, 128, 1)
    baug = np.ascontiguousarray(np.concatenate([b1a, b1b], axis=-1)).view(
        ml_dtypes.bfloat16
    )  # [E, FP, 128, 4]
    w1r = np.ascontiguousarray(
        np.concatenate([w1f[:, :FP], w1f[:, FP:], baug], axis=-1)
    )  # [E, FP, 128, W1T]
    # W2 [E, H, D] -> [E, 128p(h within k), KH*D]
    w2t = np.ascontiguousarray(
        W2.reshape(E, KH, 128, D)
        .transpose(0, 2, 1, 3)
        .reshape(E, 128, KH * D)
        .astype(ml_dtypes.bfloat16)
    )
    out = (w1r, w2t)
    _WCACHE.clear()
    _WCACHE[key] = out
    return out


def _route(x_flat, Wr):
    logits = x_flat @ np.ascontiguousarray(Wr, dtype=np.float32)  # [N, E]
    lmax = logits.max(axis=-1, keepdims=True)
    p = np.exp(logits - lmax)
    gates = p / p.sum(axis=-1, keepdims=True)
    expert = np.argmax(gates, axis=-1)
    # slot = occurrence index of each token within its expert's queue
    order = np.argsort(expert, kind="stable")
    sorted_e = expert[order]
    starts = np.searchsorted(sorted_e, np.arange(E))
    within = np.arange(N) - starts[sorted_e]
    slot = np.empty(N, np.int64)
    slot[order] = within
    kept = slot < C
    top_idx = np.zeros((C, E), np.int32)
    valid = np.zeros((C, E), np.float32)
    tok = np.arange(N, dtype=np.int32)
    top_idx[slot[kept], expert[kept]] = tok[kept]
    valid[slot[kept], expert[kept]] = 1.0
    w_ce = gates[top_idx, np.arange(E)[None, :]].astype(np.float32) * valid  # [C, E]
    n_kept = np.minimum(np.bincount(expert, minlength=E), C)  # [E]
    return gates, expert, kept, top_idx, valid, w_ce, n_kept


def kernel(x, Wr, W1, b1, W2, b2, W1f, b1f, W2f, b2f, _trace=False):
    global LAST
    _ensure_concourse()
    import ml_dtypes
    from concourse.bass_utils import run_bass_kernel_spmd

    x_flat = np.ascontiguousarray(np.asarray(x).reshape(N, D), dtype=np.float32)
    gates, expert, kept, top_idx, valid, w_ce, n_kept = _route(x_flat, np.asarray(Wr))
    w1r, w2t = _reorder_weights(np.asarray(W1), np.asarray(W2), np.asarray(b1))

    # Pair heavy experts with light ones (greedy balance); slot 0 = heavy.
    order = np.argsort(-n_kept, kind="stable")
    assign = [(int(order[i]), int(order[E - 1 - i])) for i in range(NCORES)]
    # Slot shapes: exact max routed count per slot, rounded up to 8.
    Ls = tuple(
        max(64, -(-max(int(n_kept[a[s]]) for a in assign) // 8) * 8) for s in range(EL)
    )
    L0, L1 = Ls

    nc = _get_nc(Ls)
    in_maps = []
    for c in range(NCORES):
        exps = assign[c]
        # gather + transpose tokens for each slot, chunk-major: for each
        # token chunk (c0, cn) a [128, KD*cn] block
        xparts = []
        for s, e in enumerate(exps):
            ids = top_idx[: n_kept[e], e]
            xg = np.zeros((Ls[s], D), np.float32)
            xg[: len(ids)] = x_flat[ids]
            for c0, cn in _g1_chunks(Ls[s], s == 0):
                xparts.append(
                    xg[c0 : c0 + cn]
                    .reshape(cn, KD, 128)
                    .transpose(2, 1, 0)
                    .reshape(128, KD * cn)
                    .astype(ml_dtypes.bfloat16)
                )
        pro_c = np.ascontiguousarray(
            np.concatenate([xparts[0], w1r[exps[0], 0]], axis=1)
        )
        xt_c = np.ascontiguousarray(np.concatenate(xparts[1:], axis=1))
        el = list(exps)
        in_maps.append(
            {
                "pro": pro_c,
                "xt": xt_c,
                "w1r": np.ascontiguousarray(w1r[el]),
                "w2t": np.ascontiguousarray(w2t[el]),
            }
        )
    res = run_bass_kernel_spmd(nc, in_maps, list(range(NCORES)), trace=_trace)
    LAST = res

    # Combine: gate-weight + scatter expert outputs back to token order.
    y_flat = np.zeros((N, D), np.float32)
    b2 = np.asarray(b2)
    add_b2 = bool(np.any(b2))
    for c in range(NCORES):
        yc = res.results[c]["y"]  # [D, L0+L1] bf16
        for s, e in enumerate(assign[c]):
            n = int(n_kept[e])
            ids = top_idx[:n, e]
            off = L0 if s else 0
            w = w_ce[:n, e][:, None]
            y_flat[ids] = w * yc[:, off : off + n].T.astype(np.float32)
            if add_b2:
                y_flat[ids] += w * b2[e]

    # Dense fallback for fully-dropped tokens (rare; none at typical loads).
    dropped = ~kept
    if np.any(dropped):
        xd = x_flat[dropped]
        hf = xd @ np.asarray(W1f) + np.asarray(b1f)
        gf = (hf[:, :H] / (1.0 + np.exp(-hf[:, :H]))) * hf[:, H:]
        y_flat[dropped] += FALLBACK_W * (gf @ np.asarray(W2f) + np.asarray(b2f))

    return y_flat.reshape(B, S, D)
